# revision 13
# baseline (speedup 1.0000x reference)
"""AttentionBlock (GroupNorm + single-head self-attention + proj + residual)
on 8 TRN2 NeuronCores. Data-parallel over batch: core i handles sample i.

Reference computation per sample (C=256, H=W=64, N=H*W=4096, G=32 groups):
  h    = groupnorm(x) * gamma + beta
  qkv  = w_qkv @ h + b_qkv              (1x1 conv == channel matmul)
  attn = softmax(q^T k / sqrt(C))       (N x N, never materialized in HBM)
  out  = x + w_proj @ (v @ attn^T) + b_proj

Kernel layout choices:
  - h, q, k as (C on partitions, N free) sbuf tensors (2 tiles of 128 chans).
  - v computed directly transposed (N on partitions, C free) with an extra
    ones column, so softmax denominators fall out of the same PE matmuls
    that compute attn @ v (flash-attention style, scores kept transposed).
  - scores^T tile [128 m, 512 n] -> exp on ACT -> 3 accumulating matmuls.
  - softmax never needs a max-subtraction: scores ~ N(0, 0.4^2).
  - matmuls in bf16 (1 cycle/row; f32r is a 2-pass mode on this HW). The
    residual connection dilutes attention-path rounding ~50x, so bf16 keeps
    the end-to-end rel err ~1e-3.
  - division/proj/store for block nb is emitted after block nb+1's m-loop
    (software pipelining) so PE never stalls on the softmax tail.
"""

import sys

for _p in ("/opt/trn_rl_repo", "/opt/pypackages"):
    if _p not in sys.path:
        sys.path.append(_p)

from contextlib import ExitStack

import numpy as np

import concourse.bass as bass
import concourse.tile as tile
from concourse import bacc, mybir
from concourse._compat import with_exitstack

B, C, H, W = 8, 256, 64, 64
N = H * W          # 4096
G = 32             # groups
GS = C // G        # 8 channels per group
EPS = 1e-5
P = 128
NCT = C // P       # 2 channel tiles
NBLK = 512         # attention n-block width
NB = N // NBLK     # 8
NM = N // P        # 32 m-tiles
SCALE = 1.0 / np.sqrt(np.float32(C))  # 1/16

F32 = mybir.dt.float32
BF16 = mybir.dt.bfloat16
AF = mybir.ActivationFunctionType
ALU = mybir.AluOpType


def _group_mat() -> np.ndarray:
    """A[c, c'] = 1/GS if c and c' are in the same group (within a 128-chan tile).

    out = A^T @ t averages per-channel stats over each group and broadcasts the
    group value back to every channel of the group, in one PE matmul."""
    a = np.zeros((P, P), np.float32)
    for g in range(P // GS):
        a[g * GS:(g + 1) * GS, g * GS:(g + 1) * GS] = 1.0 / GS
    return a


def _col(ap_1d, lo, hi):
    """Slice a 1-D DRAM AP into a [hi-lo, 1] AP (partition dim x 1)."""
    sl = ap_1d[lo:hi]
    return bass.AP(tensor=sl.tensor, offset=sl.offset, ap=[*sl.ap, [1, 1]])


def _bcast_rows(ap_1d, lo, hi, nrows):
    """Read ap_1d[lo:hi] identically into nrows partitions."""
    sl = ap_1d[lo:hi]
    return bass.AP(tensor=sl.tensor, offset=sl.offset, ap=[[0, nrows], *sl.ap])


@with_exitstack
def emit_kernel(ctx: ExitStack, tc: tile.TileContext, out_d, x_d, wqkvT_d,
                bqkv_d, wprojT_d, bproj_d, gamma_d, beta_d, gmat_d):
    nc = tc.nc

    big = ctx.enter_context(tc.tile_pool(name="big", bufs=1))
    small = ctx.enter_context(tc.tile_pool(name="small", bufs=1))
    work = ctx.enter_context(tc.tile_pool(name="work", bufs=3))
    work2 = ctx.enter_context(tc.tile_pool(name="work2", bufs=2))
    att_pool = ctx.enter_context(tc.tile_pool(name="att", bufs=2))
    stage = ctx.enter_context(tc.tile_pool(name="stage", bufs=3))
    xres_pool = ctx.enter_context(tc.tile_pool(name="xres", bufs=3))
    ps_s = ctx.enter_context(tc.tile_pool(name="ps_s", bufs=2, space="PSUM"))
    ps_av0 = ctx.enter_context(tc.tile_pool(name="ps_av0", bufs=2, space="PSUM"))
    ps_av1 = ctx.enter_context(tc.tile_pool(name="ps_av1", bufs=2, space="PSUM"))
    ps_sum = ctx.enter_context(tc.tile_pool(name="ps_sum", bufs=2, space="PSUM"))

    # ---- constants / weights to SBUF ----
    wq_sb = []
    wp_sb = []
    gamma_t = []
    beta_t = []
    for ct in range(NCT):
        wqf = small.tile([P, 3 * C], F32, tag=f"wqkvTf{ct}", name=f"wqf{ct}")
        nc.sync.dma_start(wqf, wqkvT_d[ct * P:(ct + 1) * P, :])
        wq = small.tile([P, 3 * C], BF16, tag=f"wqkvT{ct}", name=f"wq{ct}")
        nc.vector.tensor_copy(wq, wqf)
        wq_sb.append(wq)
        wpf = small.tile([P, C], F32, tag=f"wprojTf{ct}", name=f"wpf{ct}")
        nc.sync.dma_start(wpf, wprojT_d[ct * P:(ct + 1) * P, :])
        wp = small.tile([P, C], BF16, tag=f"wprojT{ct}", name=f"wp{ct}")
        nc.vector.tensor_copy(wp, wpf)
        wp_sb.append(wp)
        gt = small.tile([P, 1], F32, tag=f"gamma{ct}")
        nc.sync.dma_start(gt, _col(gamma_d, ct * P, (ct + 1) * P))
        gamma_t.append(gt)
        bt = small.tile([P, 1], F32, tag=f"beta{ct}")
        nc.sync.dma_start(bt, _col(beta_d, ct * P, (ct + 1) * P))
        beta_t.append(bt)

    bq_t = []
    for o in range(4):  # q, k output-channel tiles
        t = small.tile([P, 1], F32, tag=f"bq{o}")
        nc.sync.dma_start(t, _col(bqkv_d, o * P, (o + 1) * P))
        bq_t.append(t)
    bp_t = []
    for o in range(NCT):
        t = small.tile([P, 1], F32, tag=f"bp{o}")
        nc.sync.dma_start(t, _col(bproj_d, o * P, (o + 1) * P))
        bp_t.append(t)
    bv_bc = small.tile([P, C], F32, tag="bv_bc")
    nc.gpsimd.dma_start(bv_bc, _bcast_rows(bqkv_d, 2 * C, 3 * C, P))

    # gmat goes through a DVE copy so its consumer matmul waits on one engine
    # only (matmuls can carry a single sync wait through walrus codegen).
    gmat_f = small.tile([P, P], F32, tag="gmatf")
    nc.sync.dma_start(gmat_f, gmat_d[:, :])
    gmat_sb = small.tile([P, P], F32, tag="gmat")
    nc.vector.tensor_copy(gmat_sb, gmat_f)

    ones_col = small.tile([P, 1], F32, tag="ones_col")
    nc.vector.memset(ones_col, 1.0)
    eps_t = small.tile([P, 1], F32, tag="eps")
    nc.vector.memset(eps_t, float(EPS))

    # ---- load x; groupnorm stats; normalize into f32r h tiles ----
    x_sb = []
    for ct in range(NCT):
        xt = big.tile([P, N], F32, tag=f"x{ct}", name=f"x{ct}")
        for j in range(NB):
            eng = nc.sync if (j % 2 == 0) else nc.gpsimd
            eng.dma_start(xt[:, j * NBLK:(j + 1) * NBLK],
                          x_d[ct * P:(ct + 1) * P, j * NBLK:(j + 1) * NBLK])
        x_sb.append(xt)

    h_sb = []
    for ct in range(NCT):
        xt = x_sb[ct]
        stats = small.tile([P, NB, 6], F32, tag=f"bnst{ct}")
        for j in range(NB):
            nc.vector.bn_stats(stats[:, j, :], xt[:, j * NBLK:(j + 1) * NBLK])
        mv = small.tile([P, 2], F32, tag=f"mv{ct}")
        nc.vector.bn_aggr(mv, stats)
        # t = [mean_c, E[x^2]_c]
        t = small.tile([P, 2], F32, tag=f"t{ct}")
        nc.vector.tensor_copy(t[:, 0:1], mv[:, 0:1])
        nc.vector.tensor_mul(t[:, 1:2], mv[:, 0:1], mv[:, 0:1])
        nc.vector.tensor_add(t[:, 1:2], t[:, 1:2], mv[:, 1:2])
        # group-average + broadcast back to channels via PE
        psg = ps_s.tile([P, 2], F32, tag="s")
        nc.tensor.matmul(psg, lhsT=gmat_sb, rhs=t, start=True, stop=True)
        g_sb = small.tile([P, 2], F32, tag=f"g{ct}")
        nc.vector.tensor_copy(g_sb, psg)
        # scale = gamma * rsqrt(var + eps);  shift = beta - group_mean * scale
        tmp = small.tile([P, 1], F32, tag=f"tmp{ct}")
        sc = small.tile([P, 1], F32, tag=f"sc{ct}")
        sh = small.tile([P, 1], F32, tag=f"sh{ct}")
        nc.vector.tensor_mul(tmp, g_sb[:, 0:1], g_sb[:, 0:1])
        nc.vector.tensor_tensor(tmp, g_sb[:, 1:2], tmp, ALU.subtract)  # var
        nc.scalar.activation(tmp, tmp, AF.Sqrt, bias=eps_t)
        nc.vector.reciprocal(tmp, tmp)                                 # rstd
        nc.vector.tensor_mul(sc, tmp, gamma_t[ct])
        nc.vector.tensor_mul(tmp, g_sb[:, 0:1], sc)
        nc.vector.tensor_tensor(sh, beta_t[ct], tmp, ALU.subtract)
        h = big.tile([P, N], BF16, tag=f"h{ct}", name=f"h{ct}")
        nc.vector.tensor_scalar(h, xt, sc, sh, op0=ALU.mult, op1=ALU.add)
        h_sb.append(h)

    # ---- qkv projections ----
    q_sb = [big.tile([P, N], BF16, tag=f"q{ct}", name=f"q{ct}") for ct in range(NCT)]
    k_sb = [big.tile([P, N], BF16, tag=f"k{ct}", name=f"k{ct}") for ct in range(NCT)]
    for o in range(4):
        dst = q_sb[o] if o < 2 else k_sb[o - 2]
        for j in range(NB):
            ps = ps_s.tile([P, NBLK], F32, tag="s")
            for ct in range(NCT):
                nc.tensor.matmul(
                    ps,
                    lhsT=wq_sb[ct][:, o * P:(o + 1) * P],
                    rhs=h_sb[ct][:, j * NBLK:(j + 1) * NBLK],
                    start=(ct == 0), stop=(ct == NCT - 1))
            nc.vector.tensor_scalar_add(dst[:, j * NBLK:(j + 1) * NBLK], ps, bq_t[o])

    # v, already transposed: vt[m*128+p, c] = v[c, m*128+p]; col 256 = ones.
    # Reuses the (dead) x tiles' SBUF via shared tags; x is re-read from DRAM
    # later for the residual.
    vt_lo = big.tile([P, NM // 2, C + 1], BF16, tag="x0", name="vt_lo")
    vt_hi = big.tile([P, NM // 2, C + 1], BF16, tag="x1", name="vt_hi")

    def vt(m):
        return vt_lo[:, m] if m < NM // 2 else vt_hi[:, m - NM // 2]

    for m in range(NM):
        ps = ps_s.tile([P, C], F32, tag="s")
        for ct in range(NCT):
            nc.tensor.matmul(
                ps,
                lhsT=h_sb[ct][:, m * P:(m + 1) * P],
                rhs=wq_sb[ct][:, 2 * C:3 * C],
                start=(ct == 0), stop=(ct == NCT - 1))
        nc.vector.tensor_add(vt(m)[:, 0:C], ps, bv_bc)
        nc.vector.tensor_copy(vt(m)[:, C:C + 1], ones_col)

    # ---- attention + proj + residual, per 512-column block ----
    def emit_div_proj(pend):
        pav0, pav1, psum, nb = pend
        nsl = slice(nb * NBLK, (nb + 1) * NBLK)
        # 1/rowsum; broadcast partition 0 to all 128 partitions via DMA
        recip = work2.tile([1, NBLK], F32, tag="recip")
        nc.vector.reciprocal(recip, psum)
        bc_sb = work2.tile([P, NBLK], F32, tag="bc")
        nc.gpsimd.partition_broadcast(bc_sb, recip)
        att = []
        for ct, pav in ((0, pav0), (1, pav1)):
            a = att_pool.tile([P, NBLK], BF16, tag=f"att{ct}", name=f"att{ct}")
            nc.vector.tensor_mul(a, pav, bc_sb)
            att.append(a)
        for o in range(NCT):
            pp = ps_s.tile([P, NBLK], F32, tag="s")
            for ct in range(NCT):
                nc.tensor.matmul(
                    pp, lhsT=wp_sb[ct][:, o * P:(o + 1) * P],
                    rhs=att[ct], start=(ct == 0), stop=(ct == NCT - 1))
            xres = xres_pool.tile([P, NBLK], F32, tag="xr")
            nc.sync.dma_start(xres, x_d[o * P:(o + 1) * P, nsl])
            st = stage.tile([P, NBLK], F32, tag="st")
            nc.vector.tensor_scalar_add(st, pp, bp_t[o])
            nc.vector.tensor_add(st, st, xres)
            nc.sync.dma_start(out_d[o * P:(o + 1) * P, nsl], st)

    pend = None
    for nb in range(NB):
        nsl = slice(nb * NBLK, (nb + 1) * NBLK)
        qs = [q_sb[ct][:, nsl] for ct in range(NCT)]  # noqa
        pav0 = ps_av0.tile([P, NBLK], F32, tag="av0")
        pav1 = ps_av1.tile([P, NBLK], F32, tag="av1")
        psum = ps_sum.tile([1, NBLK], F32, tag="sum")
        for m in range(NM):
            ps = ps_s.tile([P, NBLK], F32, tag="s")
            for ct in range(NCT):
                nc.tensor.matmul(
                    ps, lhsT=k_sb[ct][:, m * P:(m + 1) * P],
                    rhs=qs[ct], start=(ct == 0), stop=(ct == NCT - 1))
            e = work.tile([P, NBLK], BF16, tag="e")
            nc.scalar.activation(e, ps, AF.Exp, scale=float(SCALE))
            er = e[:]
            first, last = (m == 0), (m == NM - 1)
            vtm = vt(m)
            nc.tensor.matmul(pav0, lhsT=vtm[:, 0:P], rhs=er,
                             start=first, stop=last)
            nc.tensor.matmul(pav1, lhsT=vtm[:, P:2 * P], rhs=er,
                             start=first, stop=last)
            nc.tensor.matmul(psum, lhsT=vtm[:, 2 * P:2 * P + 1],
                             rhs=er, start=first, stop=last)
        if pend is not None:
            emit_div_proj(pend)
        pend = (pav0, pav1, psum, nb)
    emit_div_proj(pend)


def build_nc() -> bass.Bass:
    nc = bacc.Bacc("TRN2", target_bir_lowering=False, debug=False)
    x = nc.dram_tensor("x", [C, N], F32, kind="ExternalInput")
    wqkvT = nc.dram_tensor("wqkvT", [C, 3 * C], F32, kind="ExternalInput")
    bqkv = nc.dram_tensor("bqkv", [3 * C], F32, kind="ExternalInput")
    wprojT = nc.dram_tensor("wprojT", [C, C], F32, kind="ExternalInput")
    bproj = nc.dram_tensor("bproj", [C], F32, kind="ExternalInput")
    gamma = nc.dram_tensor("gamma", [C], F32, kind="ExternalInput")
    beta = nc.dram_tensor("beta", [C], F32, kind="ExternalInput")
    gmat = nc.dram_tensor("gmat", [P, P], F32, kind="ExternalInput")
    out = nc.dram_tensor("out", [C, N], F32, kind="ExternalOutput")
    with tile.TileContext(nc) as tc:
        emit_kernel(tc, out.ap(), x.ap(), wqkvT.ap(), bqkv.ap(), wprojT.ap(),
                    bproj.ap(), gamma.ap(), beta.ap(), gmat.ap())
    nc.compile()
    return nc


_NC_CACHE: list = []


def _in_maps(x, gamma, beta, w_qkv, b_qkv, w_proj, b_proj):
    f = lambda a: np.ascontiguousarray(np.asarray(a, dtype=np.float32))
    xs = f(x).reshape(B, C, N)
    base = {
        "wqkvT": f(np.asarray(w_qkv, dtype=np.float32).T),
        "bqkv": f(b_qkv),
        "wprojT": f(np.asarray(w_proj, dtype=np.float32).T),
        "bproj": f(b_proj),
        "gamma": f(gamma),
        "beta": f(beta),
        "gmat": _group_mat(),
    }
    return [{**base, "x": np.ascontiguousarray(xs[i])} for i in range(B)]


def run_spmd(x, gamma, beta, w_qkv, b_qkv, w_proj, b_proj, **kwargs):
    from concourse.bass_utils import run_bass_kernel_spmd

    if not _NC_CACHE:
        _NC_CACHE.append(build_nc())
    nc = _NC_CACHE[0]
    maps = _in_maps(x, gamma, beta, w_qkv, b_qkv, w_proj, b_proj)
    res = run_bass_kernel_spmd(nc, maps, core_ids=list(range(B)), **kwargs)
    out = np.stack([res.results[i]["out"] for i in range(B)])
    return out.reshape(B, C, H, W), res


def kernel(x, gamma, beta, w_qkv, b_qkv, w_proj, b_proj) -> np.ndarray:
    out, _ = run_spmd(x, gamma, beta, w_qkv, b_qkv, w_proj, b_proj)
    return out


# revision 14
# speedup vs baseline: 1.1301x; 1.1301x over previous
"""AttentionBlock (GroupNorm + single-head self-attention + proj + residual)
on 8 TRN2 NeuronCores. Data-parallel over batch: core i handles sample i.

Reference computation per sample (C=256, H=W=64, N=H*W=4096, G=32 groups):
  h    = groupnorm(x) * gamma + beta
  qkv  = w_qkv @ h + b_qkv              (1x1 conv == channel matmul)
  attn = softmax(q^T k / sqrt(C))       (N x N, never materialized in HBM)
  out  = x + w_proj @ (v @ attn^T) + b_proj

Kernel layout choices:
  - h, q, k as (C on partitions, N free) sbuf tensors (2 tiles of 128 chans).
  - v computed directly transposed (N on partitions, C free) with an extra
    ones column, so softmax denominators fall out of the same PE matmuls
    that compute attn @ v (flash-attention style, scores kept transposed).
  - scores^T tile [128 m, 512 n] -> exp on ACT -> 3 accumulating matmuls.
  - softmax never needs a max-subtraction: scores ~ N(0, 0.4^2).
  - matmuls in bf16 (1 cycle/row; f32r is a 2-pass mode on this HW). The
    residual connection dilutes attention-path rounding ~50x, so bf16 keeps
    the end-to-end rel err ~1e-3.
  - division/proj/store for block nb is emitted after block nb+1's m-loop
    (software pipelining) so PE never stalls on the softmax tail.
"""

import sys

for _p in ("/opt/trn_rl_repo", "/opt/pypackages"):
    if _p not in sys.path:
        sys.path.append(_p)

from contextlib import ExitStack

import numpy as np

import concourse.bass as bass
import concourse.tile as tile
from concourse import bacc, mybir
from concourse._compat import with_exitstack

B, C, H, W = 8, 256, 64, 64
N = H * W          # 4096
G = 32             # groups
GS = C // G        # 8 channels per group
EPS = 1e-5
P = 128
NCT = C // P       # 2 channel tiles
NBLK = 512         # attention n-block width
NB = N // NBLK     # 8
NM = N // P        # 32 m-tiles
SCALE = 1.0 / np.sqrt(np.float32(C))  # 1/16

F32 = mybir.dt.float32
BF16 = mybir.dt.bfloat16
AF = mybir.ActivationFunctionType
ALU = mybir.AluOpType


def _group_mat() -> np.ndarray:
    """A[c, c'] = 1/GS if c and c' are in the same group (within a 128-chan tile).

    out = A^T @ t averages per-channel stats over each group and broadcasts the
    group value back to every channel of the group, in one PE matmul."""
    a = np.zeros((P, P), np.float32)
    for g in range(P // GS):
        a[g * GS:(g + 1) * GS, g * GS:(g + 1) * GS] = 1.0 / GS
    return a


def _col(ap_1d, lo, hi):
    """Slice a 1-D DRAM AP into a [hi-lo, 1] AP (partition dim x 1)."""
    sl = ap_1d[lo:hi]
    return bass.AP(tensor=sl.tensor, offset=sl.offset, ap=[*sl.ap, [1, 1]])


def _bcast_rows(ap_1d, lo, hi, nrows):
    """Read ap_1d[lo:hi] identically into nrows partitions."""
    sl = ap_1d[lo:hi]
    return bass.AP(tensor=sl.tensor, offset=sl.offset, ap=[[0, nrows], *sl.ap])


@with_exitstack
def emit_kernel(ctx: ExitStack, tc: tile.TileContext, out_d, x_d, wqkvT_d,
                bqkv_d, wprojT_d, bproj_d, gamma_d, beta_d, gmat_d):
    nc = tc.nc

    big = ctx.enter_context(tc.tile_pool(name="big", bufs=1))
    small = ctx.enter_context(tc.tile_pool(name="small", bufs=1))
    work = ctx.enter_context(tc.tile_pool(name="work", bufs=3))
    work2 = ctx.enter_context(tc.tile_pool(name="work2", bufs=2))
    att_pool = ctx.enter_context(tc.tile_pool(name="att", bufs=2))
    stage = ctx.enter_context(tc.tile_pool(name="stage", bufs=3))
    xres_pool = ctx.enter_context(tc.tile_pool(name="xres", bufs=3))
    ps_s = ctx.enter_context(tc.tile_pool(name="ps_s", bufs=3, space="PSUM"))
    ps_av0 = ctx.enter_context(tc.tile_pool(name="ps_av0", bufs=2, space="PSUM"))
    ps_av1 = ctx.enter_context(tc.tile_pool(name="ps_av1", bufs=2, space="PSUM"))
    ps_sum = ctx.enter_context(tc.tile_pool(name="ps_sum", bufs=1, space="PSUM"))

    # ---- constants / weights to SBUF ----
    wq_sb = []
    wp_sb = []
    gamma_t = []
    beta_t = []
    for ct in range(NCT):
        wqf = small.tile([P, 3 * C], F32, tag=f"wqkvTf{ct}", name=f"wqf{ct}")
        nc.sync.dma_start(wqf, wqkvT_d[ct * P:(ct + 1) * P, :])
        wq = small.tile([P, 3 * C], BF16, tag=f"wqkvT{ct}", name=f"wq{ct}")
        nc.vector.tensor_copy(wq, wqf)
        wq_sb.append(wq)
        wpf = small.tile([P, C], F32, tag=f"wprojTf{ct}", name=f"wpf{ct}")
        nc.sync.dma_start(wpf, wprojT_d[ct * P:(ct + 1) * P, :])
        wp = small.tile([P, C], BF16, tag=f"wprojT{ct}", name=f"wp{ct}")
        nc.vector.tensor_copy(wp, wpf)
        wp_sb.append(wp)
        gt = small.tile([P, 1], F32, tag=f"gamma{ct}")
        nc.sync.dma_start(gt, _col(gamma_d, ct * P, (ct + 1) * P))
        gamma_t.append(gt)
        bt = small.tile([P, 1], F32, tag=f"beta{ct}")
        nc.sync.dma_start(bt, _col(beta_d, ct * P, (ct + 1) * P))
        beta_t.append(bt)

    bq_t = []
    for o in range(4):  # q, k output-channel tiles
        t = small.tile([P, 1], F32, tag=f"bq{o}")
        nc.sync.dma_start(t, _col(bqkv_d, o * P, (o + 1) * P))
        bq_t.append(t)
    bp_t = []
    for o in range(NCT):
        t = small.tile([P, 1], F32, tag=f"bp{o}")
        nc.sync.dma_start(t, _col(bproj_d, o * P, (o + 1) * P))
        bp_t.append(t)
    bv_bc = small.tile([P, C], F32, tag="bv_bc")
    nc.gpsimd.dma_start(bv_bc, _bcast_rows(bqkv_d, 2 * C, 3 * C, P))

    # gmat goes through a DVE copy so its consumer matmul waits on one engine
    # only (matmuls can carry a single sync wait through walrus codegen).
    gmat_f = small.tile([P, P], F32, tag="gmatf")
    nc.sync.dma_start(gmat_f, gmat_d[:, :])
    gmat_sb = small.tile([P, P], F32, tag="gmat")
    nc.vector.tensor_copy(gmat_sb, gmat_f)

    ones_col = small.tile([P, 1], F32, tag="ones_col")
    nc.vector.memset(ones_col, 1.0)
    eps_t = small.tile([P, 1], F32, tag="eps")
    nc.vector.memset(eps_t, float(EPS))

    # ---- load x; groupnorm stats; normalize into f32r h tiles ----
    x_sb = []
    for ct in range(NCT):
        xt = big.tile([P, N], F32, tag=f"x{ct}", name=f"x{ct}")
        for j in range(NB):
            eng = nc.sync if (j % 2 == 0) else nc.gpsimd
            eng.dma_start(xt[:, j * NBLK:(j + 1) * NBLK],
                          x_d[ct * P:(ct + 1) * P, j * NBLK:(j + 1) * NBLK])
        x_sb.append(xt)

    h_sb = []
    for ct in range(NCT):
        xt = x_sb[ct]
        stats = small.tile([P, NB, 6], F32, tag=f"bnst{ct}")
        for j in range(NB):
            nc.vector.bn_stats(stats[:, j, :], xt[:, j * NBLK:(j + 1) * NBLK])
        mv = small.tile([P, 2], F32, tag=f"mv{ct}")
        nc.vector.bn_aggr(mv, stats)
        # t = [mean_c, E[x^2]_c]
        t = small.tile([P, 2], F32, tag=f"t{ct}")
        nc.vector.tensor_copy(t[:, 0:1], mv[:, 0:1])
        nc.vector.tensor_mul(t[:, 1:2], mv[:, 0:1], mv[:, 0:1])
        nc.vector.tensor_add(t[:, 1:2], t[:, 1:2], mv[:, 1:2])
        # group-average + broadcast back to channels via PE
        psg = ps_s.tile([P, 2], F32, tag="s")
        nc.tensor.matmul(psg, lhsT=gmat_sb, rhs=t, start=True, stop=True)
        g_sb = small.tile([P, 2], F32, tag=f"g{ct}")
        nc.vector.tensor_copy(g_sb, psg)
        # scale = gamma * rsqrt(var + eps);  shift = beta - group_mean * scale
        tmp = small.tile([P, 1], F32, tag=f"tmp{ct}")
        sc = small.tile([P, 1], F32, tag=f"sc{ct}")
        sh = small.tile([P, 1], F32, tag=f"sh{ct}")
        nc.vector.tensor_mul(tmp, g_sb[:, 0:1], g_sb[:, 0:1])
        nc.vector.tensor_tensor(tmp, g_sb[:, 1:2], tmp, ALU.subtract)  # var
        nc.scalar.activation(tmp, tmp, AF.Sqrt, bias=eps_t)
        nc.vector.reciprocal(tmp, tmp)                                 # rstd
        nc.vector.tensor_mul(sc, tmp, gamma_t[ct])
        nc.vector.tensor_mul(tmp, g_sb[:, 0:1], sc)
        nc.vector.tensor_tensor(sh, beta_t[ct], tmp, ALU.subtract)
        h = big.tile([P, N], BF16, tag=f"h{ct}", name=f"h{ct}")
        nc.vector.tensor_scalar(h, xt, sc, sh, op0=ALU.mult, op1=ALU.add)
        h_sb.append(h)

    # ---- qkv projections ----
    q_sb = [big.tile([P, N], BF16, tag=f"q{ct}", name=f"q{ct}") for ct in range(NCT)]
    k_sb = [big.tile([P, N], BF16, tag=f"k{ct}", name=f"k{ct}") for ct in range(NCT)]
    for o in range(4):
        dst = q_sb[o] if o < 2 else k_sb[o - 2]
        for j in range(NB):
            ps = ps_s.tile([P, NBLK], F32, tag="s")
            for ct in range(NCT):
                nc.tensor.matmul(
                    ps,
                    lhsT=wq_sb[ct][:, o * P:(o + 1) * P],
                    rhs=h_sb[ct][:, j * NBLK:(j + 1) * NBLK],
                    start=(ct == 0), stop=(ct == NCT - 1))
            nc.vector.tensor_scalar_add(dst[:, j * NBLK:(j + 1) * NBLK], ps, bq_t[o])

    # v, already transposed: vt[m*128+p, c] = v[c, m*128+p]; col 256 = ones.
    # Reuses the (dead) x tiles' SBUF via shared tags; x is re-read from DRAM
    # later for the residual.
    vt_lo = big.tile([P, NM // 2, C + 1], BF16, tag="x0", name="vt_lo")
    vt_hi = big.tile([P, NM // 2, C + 1], BF16, tag="x1", name="vt_hi")

    def vt(m):
        return vt_lo[:, m] if m < NM // 2 else vt_hi[:, m - NM // 2]

    for m in range(NM):
        ps = ps_s.tile([P, C], F32, tag="s")
        for ct in range(NCT):
            nc.tensor.matmul(
                ps,
                lhsT=h_sb[ct][:, m * P:(m + 1) * P],
                rhs=wq_sb[ct][:, 2 * C:3 * C],
                start=(ct == 0), stop=(ct == NCT - 1))
        nc.vector.tensor_add(vt(m)[:, 0:C], ps, bv_bc)
        nc.vector.tensor_copy(vt(m)[:, C:C + 1], ones_col)

    # ---- attention + proj + residual, per 512-column block ----
    def emit_div_proj(pend):
        pav0, pav1, psum, nb = pend
        nsl = slice(nb * NBLK, (nb + 1) * NBLK)
        # 1/rowsum; broadcast partition 0 to all 128 partitions via DMA
        recip = work2.tile([1, NBLK], F32, tag="recip")
        nc.vector.reciprocal(recip, psum)
        bc_sb = work2.tile([P, NBLK], F32, tag="bc")
        nc.gpsimd.partition_broadcast(bc_sb, recip)
        att = []
        for ct, pav in ((0, pav0), (1, pav1)):
            a = att_pool.tile([P, NBLK], BF16, tag=f"att{ct}", name=f"att{ct}")
            nc.vector.tensor_mul(a, pav, bc_sb)
            att.append(a)
        for o in range(NCT):
            pp = ps_s.tile([P, NBLK], F32, tag="s")
            for ct in range(NCT):
                nc.tensor.matmul(
                    pp, lhsT=wp_sb[ct][:, o * P:(o + 1) * P],
                    rhs=att[ct], start=(ct == 0), stop=(ct == NCT - 1))
            xres = xres_pool.tile([P, NBLK], F32, tag="xr")
            nc.sync.dma_start(xres, x_d[o * P:(o + 1) * P, nsl])
            st = stage.tile([P, NBLK], F32, tag="st")
            nc.vector.tensor_scalar_add(st, pp, bp_t[o])
            nc.vector.tensor_add(st, st, xres)
            nc.sync.dma_start(out_d[o * P:(o + 1) * P, nsl], st)

    # m-loop with 2-deep scores lookahead: PE program order is
    # s(0) s(1) [div/proj of prev block] s(2) av(0) s(3) av(1) ... so the
    # exp(m) ACT latency is hidden behind s(m+1)/s(m+2) instead of stalling
    # the av(m) matmuls (matmul waits break the PE fill/drain overlap).
    pend = None
    for nb in range(NB):
        nsl = slice(nb * NBLK, (nb + 1) * NBLK)
        qs = [q_sb[ct][:, nsl] for ct in range(NCT)]  # noqa
        pav0 = ps_av0.tile([P, NBLK], F32, tag="av0")
        pav1 = ps_av1.tile([P, NBLK], F32, tag="av1")

        ps_m = {}
        e_m = {}

        def emit_scores(m, qs=qs):
            ps = ps_s.tile([P, NBLK], F32, tag="s")
            for ct in range(NCT):
                nc.tensor.matmul(
                    ps, lhsT=k_sb[ct][:, m * P:(m + 1) * P],
                    rhs=qs[ct], start=(ct == 0), stop=(ct == NCT - 1))
            ps_m[m] = ps

        emit_scores(0)
        emit_scores(1)
        if pend is not None:
            emit_div_proj(pend)
        psum = ps_sum.tile([1, NBLK], F32, tag="sum")
        for m in range(NM):
            if m + 2 < NM:
                emit_scores(m + 2)
            e = work.tile([P, NBLK], BF16, tag="e")
            nc.scalar.activation(e, ps_m.pop(m), AF.Exp, scale=float(SCALE))
            er = e[:]
            first, last = (m == 0), (m == NM - 1)
            vtm = vt(m)
            nc.tensor.matmul(pav0, lhsT=vtm[:, 0:P], rhs=er,
                             start=first, stop=last)
            nc.tensor.matmul(pav1, lhsT=vtm[:, P:2 * P], rhs=er,
                             start=first, stop=last)
            nc.tensor.matmul(psum, lhsT=vtm[:, 2 * P:2 * P + 1],
                             rhs=er, start=first, stop=last)
        pend = (pav0, pav1, psum, nb)
    emit_div_proj(pend)


def build_nc() -> bass.Bass:
    nc = bacc.Bacc("TRN2", target_bir_lowering=False, debug=False)
    x = nc.dram_tensor("x", [C, N], F32, kind="ExternalInput")
    wqkvT = nc.dram_tensor("wqkvT", [C, 3 * C], F32, kind="ExternalInput")
    bqkv = nc.dram_tensor("bqkv", [3 * C], F32, kind="ExternalInput")
    wprojT = nc.dram_tensor("wprojT", [C, C], F32, kind="ExternalInput")
    bproj = nc.dram_tensor("bproj", [C], F32, kind="ExternalInput")
    gamma = nc.dram_tensor("gamma", [C], F32, kind="ExternalInput")
    beta = nc.dram_tensor("beta", [C], F32, kind="ExternalInput")
    gmat = nc.dram_tensor("gmat", [P, P], F32, kind="ExternalInput")
    out = nc.dram_tensor("out", [C, N], F32, kind="ExternalOutput")
    with tile.TileContext(nc) as tc:
        emit_kernel(tc, out.ap(), x.ap(), wqkvT.ap(), bqkv.ap(), wprojT.ap(),
                    bproj.ap(), gamma.ap(), beta.ap(), gmat.ap())
    nc.compile()
    return nc


_NC_CACHE: list = []


def _in_maps(x, gamma, beta, w_qkv, b_qkv, w_proj, b_proj):
    f = lambda a: np.ascontiguousarray(np.asarray(a, dtype=np.float32))
    xs = f(x).reshape(B, C, N)
    base = {
        "wqkvT": f(np.asarray(w_qkv, dtype=np.float32).T),
        "bqkv": f(b_qkv),
        "wprojT": f(np.asarray(w_proj, dtype=np.float32).T),
        "bproj": f(b_proj),
        "gamma": f(gamma),
        "beta": f(beta),
        "gmat": _group_mat(),
    }
    return [{**base, "x": np.ascontiguousarray(xs[i])} for i in range(B)]


def run_spmd(x, gamma, beta, w_qkv, b_qkv, w_proj, b_proj, **kwargs):
    from concourse.bass_utils import run_bass_kernel_spmd

    if not _NC_CACHE:
        _NC_CACHE.append(build_nc())
    nc = _NC_CACHE[0]
    maps = _in_maps(x, gamma, beta, w_qkv, b_qkv, w_proj, b_proj)
    res = run_bass_kernel_spmd(nc, maps, core_ids=list(range(B)), **kwargs)
    out = np.stack([res.results[i]["out"] for i in range(B)])
    return out.reshape(B, C, H, W), res


def kernel(x, gamma, beta, w_qkv, b_qkv, w_proj, b_proj) -> np.ndarray:
    out, _ = run_spmd(x, gamma, beta, w_qkv, b_qkv, w_proj, b_proj)
    return out


# revision 15
# speedup vs baseline: 1.1566x; 1.0234x over previous
"""AttentionBlock (GroupNorm + single-head self-attention + proj + residual)
on 8 TRN2 NeuronCores. Data-parallel over batch: core i handles sample i.

Reference computation per sample (C=256, H=W=64, N=H*W=4096, G=32 groups):
  h    = groupnorm(x) * gamma + beta
  qkv  = w_qkv @ h + b_qkv              (1x1 conv == channel matmul)
  attn = softmax(q^T k / sqrt(C))       (N x N, never materialized in HBM)
  out  = x + w_proj @ (v @ attn^T) + b_proj

Kernel layout choices:
  - h, q, k as (C on partitions, N free) sbuf tensors (2 tiles of 128 chans).
  - v computed directly transposed (N on partitions, C free) with an extra
    ones column, so softmax denominators fall out of the same PE matmuls
    that compute attn @ v (flash-attention style, scores kept transposed).
  - scores^T tile [128 m, 512 n] -> exp on ACT -> 3 accumulating matmuls.
  - softmax never needs a max-subtraction: scores ~ N(0, 0.4^2).
  - matmuls in bf16 (1 cycle/row; f32r is a 2-pass mode on this HW). The
    residual connection dilutes attention-path rounding ~50x, so bf16 keeps
    the end-to-end rel err ~1e-3.
  - division/proj/store for block nb is emitted after block nb+1's m-loop
    (software pipelining) so PE never stalls on the softmax tail.
"""

import sys

for _p in ("/opt/trn_rl_repo", "/opt/pypackages"):
    if _p not in sys.path:
        sys.path.append(_p)

from contextlib import ExitStack

import numpy as np

import concourse.bass as bass
import concourse.tile as tile
from concourse import bacc, mybir
from concourse._compat import with_exitstack

B, C, H, W = 8, 256, 64, 64
N = H * W          # 4096
G = 32             # groups
GS = C // G        # 8 channels per group
EPS = 1e-5
P = 128
NCT = C // P       # 2 channel tiles
NBLK = 512         # attention n-block width
NB = N // NBLK     # 8
NM = N // P        # 32 m-tiles
SCALE = 1.0 / np.sqrt(np.float32(C))  # 1/16
WARMUP_MM = 80     # fp32 gmat matmuls to keep PE's HAM clock-gate warm

F32 = mybir.dt.float32
BF16 = mybir.dt.bfloat16
AF = mybir.ActivationFunctionType
ALU = mybir.AluOpType


def _group_mat() -> np.ndarray:
    """A[c, c'] = 1/GS if c and c' are in the same group (within a 128-chan tile).

    out = A^T @ t averages per-channel stats over each group and broadcasts the
    group value back to every channel of the group, in one PE matmul."""
    a = np.zeros((P, P), np.float32)
    for g in range(P // GS):
        a[g * GS:(g + 1) * GS, g * GS:(g + 1) * GS] = 1.0 / GS
    return a


def _col(ap_1d, lo, hi):
    """Slice a 1-D DRAM AP into a [hi-lo, 1] AP (partition dim x 1)."""
    sl = ap_1d[lo:hi]
    return bass.AP(tensor=sl.tensor, offset=sl.offset, ap=[*sl.ap, [1, 1]])


def _bcast_rows(ap_1d, lo, hi, nrows):
    """Read ap_1d[lo:hi] identically into nrows partitions."""
    sl = ap_1d[lo:hi]
    return bass.AP(tensor=sl.tensor, offset=sl.offset, ap=[[0, nrows], *sl.ap])


@with_exitstack
def emit_kernel(ctx: ExitStack, tc: tile.TileContext, out_d, x_d, wqkvT_d,
                bqkv_d, wprojT_d, bproj_d, gamma_d, beta_d, gmat_d):
    nc = tc.nc

    big = ctx.enter_context(tc.tile_pool(name="big", bufs=1))
    small = ctx.enter_context(tc.tile_pool(name="small", bufs=1))
    work = ctx.enter_context(tc.tile_pool(name="work", bufs=3))
    work2 = ctx.enter_context(tc.tile_pool(name="work2", bufs=2))
    att_pool = ctx.enter_context(tc.tile_pool(name="att", bufs=2))
    stage = ctx.enter_context(tc.tile_pool(name="stage", bufs=3))
    xres_pool = ctx.enter_context(tc.tile_pool(name="xres", bufs=3))
    ps_s = ctx.enter_context(tc.tile_pool(name="ps_s", bufs=3, space="PSUM"))
    ps_av0 = ctx.enter_context(tc.tile_pool(name="ps_av0", bufs=2, space="PSUM"))
    ps_av1 = ctx.enter_context(tc.tile_pool(name="ps_av1", bufs=2, space="PSUM"))
    ps_sum = ctx.enter_context(tc.tile_pool(name="ps_sum", bufs=1, space="PSUM"))

    # ---- gmat first: its DVE copy feeds PE warmup matmuls that keep the
    # HAM clock-gate warm while x loads / groupnorm stats run ----
    gmat_f = small.tile([P, P], F32, tag="gmatf")
    nc.sync.dma_start(gmat_f, gmat_d[:, :])
    gmat_sb = small.tile([P, P], F32, tag="gmat")
    nc.vector.tensor_copy(gmat_sb, gmat_f)
    for w in range(WARMUP_MM):
        pw = ps_s.tile([P, P], F32, tag="s", name=f"warm{w}")
        nc.tensor.matmul(pw, lhsT=gmat_sb, rhs=gmat_sb, start=True, stop=True)

    # ---- constants / weights to SBUF ----
    wq_sb = []
    wp_sb = []
    gamma_t = []
    beta_t = []
    for ct in range(NCT):
        wqf = small.tile([P, 3 * C], F32, tag=f"wqkvTf{ct}", name=f"wqf{ct}")
        nc.sync.dma_start(wqf, wqkvT_d[ct * P:(ct + 1) * P, :])
        wq = small.tile([P, 3 * C], BF16, tag=f"wqkvT{ct}", name=f"wq{ct}")
        nc.vector.tensor_copy(wq, wqf)
        wq_sb.append(wq)
        wpf = small.tile([P, C], F32, tag=f"wprojTf{ct}", name=f"wpf{ct}")
        nc.sync.dma_start(wpf, wprojT_d[ct * P:(ct + 1) * P, :])
        wp = small.tile([P, C], BF16, tag=f"wprojT{ct}", name=f"wp{ct}")
        nc.vector.tensor_copy(wp, wpf)
        wp_sb.append(wp)
        gt = small.tile([P, 1], F32, tag=f"gamma{ct}")
        nc.sync.dma_start(gt, _col(gamma_d, ct * P, (ct + 1) * P))
        gamma_t.append(gt)
        bt = small.tile([P, 1], F32, tag=f"beta{ct}")
        nc.sync.dma_start(bt, _col(beta_d, ct * P, (ct + 1) * P))
        beta_t.append(bt)

    bq_t = []
    for o in range(4):  # q, k output-channel tiles
        t = small.tile([P, 1], F32, tag=f"bq{o}")
        nc.sync.dma_start(t, _col(bqkv_d, o * P, (o + 1) * P))
        bq_t.append(t)
    bp_t = []
    for o in range(NCT):
        t = small.tile([P, 1], F32, tag=f"bp{o}")
        nc.sync.dma_start(t, _col(bproj_d, o * P, (o + 1) * P))
        bp_t.append(t)
    bv_bc = small.tile([P, C], F32, tag="bv_bc")
    nc.gpsimd.dma_start(bv_bc, _bcast_rows(bqkv_d, 2 * C, 3 * C, P))

    ones_col = small.tile([P, 1], F32, tag="ones_col")
    nc.vector.memset(ones_col, 1.0)
    eps_t = small.tile([P, 1], F32, tag="eps")
    nc.vector.memset(eps_t, float(EPS))

    # ---- load x (ct0 on the HW DGE queue, ct1 on the SW DGE queue, in
    # parallel); groupnorm stats interleaved with chunk arrival ----
    x_sb = []
    stats_t = []
    for ct in range(NCT):
        xt = big.tile([P, N], F32, tag=f"x{ct}", name=f"x{ct}")
        x_sb.append(xt)
        stats_t.append(small.tile([P, NB, 6], F32, tag=f"bnst{ct}",
                                  name=f"bnst{ct}"))
    for j in range(NB):
        for ct in range(NCT):
            eng = nc.sync if ct == 0 else nc.gpsimd
            eng.dma_start(x_sb[ct][:, j * NBLK:(j + 1) * NBLK],
                          x_d[ct * P:(ct + 1) * P, j * NBLK:(j + 1) * NBLK])
    for j in range(NB):
        for ct in range(NCT):
            nc.vector.bn_stats(stats_t[ct][:, j, :],
                               x_sb[ct][:, j * NBLK:(j + 1) * NBLK])

    h_sb = []
    for ct in range(NCT):
        xt = x_sb[ct]
        mv = small.tile([P, 2], F32, tag=f"mv{ct}")
        nc.vector.bn_aggr(mv, stats_t[ct])
        # t = [mean_c, E[x^2]_c]
        t = small.tile([P, 2], F32, tag=f"t{ct}")
        nc.vector.tensor_copy(t[:, 0:1], mv[:, 0:1])
        nc.vector.tensor_mul(t[:, 1:2], mv[:, 0:1], mv[:, 0:1])
        nc.vector.tensor_add(t[:, 1:2], t[:, 1:2], mv[:, 1:2])
        # group-average + broadcast back to channels via PE
        psg = ps_s.tile([P, 2], F32, tag="s")
        nc.tensor.matmul(psg, lhsT=gmat_sb, rhs=t, start=True, stop=True)
        g_sb = small.tile([P, 2], F32, tag=f"g{ct}")
        nc.vector.tensor_copy(g_sb, psg)
        # scale = gamma * rsqrt(var + eps);  shift = beta - group_mean * scale
        tmp = small.tile([P, 1], F32, tag=f"tmp{ct}")
        sc = small.tile([P, 1], F32, tag=f"sc{ct}")
        sh = small.tile([P, 1], F32, tag=f"sh{ct}")
        nc.vector.tensor_mul(tmp, g_sb[:, 0:1], g_sb[:, 0:1])
        nc.vector.tensor_tensor(tmp, g_sb[:, 1:2], tmp, ALU.subtract)  # var
        nc.scalar.activation(tmp, tmp, AF.Sqrt, bias=eps_t)
        nc.vector.reciprocal(tmp, tmp)                                 # rstd
        nc.vector.tensor_mul(sc, tmp, gamma_t[ct])
        nc.vector.tensor_mul(tmp, g_sb[:, 0:1], sc)
        nc.vector.tensor_tensor(sh, beta_t[ct], tmp, ALU.subtract)
        h = big.tile([P, N], BF16, tag=f"h{ct}", name=f"h{ct}")
        for j in range(4):
            csl = slice(j * (N // 4), (j + 1) * (N // 4))
            nc.vector.tensor_scalar(h[:, csl], xt[:, csl], sc, sh,
                                    op0=ALU.mult, op1=ALU.add)
        h_sb.append(h)

    # ---- qkv projections ----
    q_sb = [big.tile([P, N], BF16, tag=f"q{ct}", name=f"q{ct}") for ct in range(NCT)]
    k_sb = [big.tile([P, N], BF16, tag=f"k{ct}", name=f"k{ct}") for ct in range(NCT)]
    for o in range(4):
        dst = q_sb[o] if o < 2 else k_sb[o - 2]
        for j in range(NB):
            ps = ps_s.tile([P, NBLK], F32, tag="s")
            for ct in range(NCT):
                nc.tensor.matmul(
                    ps,
                    lhsT=wq_sb[ct][:, o * P:(o + 1) * P],
                    rhs=h_sb[ct][:, j * NBLK:(j + 1) * NBLK],
                    start=(ct == 0), stop=(ct == NCT - 1))
            nc.vector.tensor_scalar_add(dst[:, j * NBLK:(j + 1) * NBLK], ps, bq_t[o])

    # v, already transposed: vt[m*128+p, c] = v[c, m*128+p]; col 256 = ones.
    # Reuses the (dead) x tiles' SBUF via shared tags; x is re-read from DRAM
    # later for the residual.
    vt_lo = big.tile([P, NM // 2, C + 1], BF16, tag="x0", name="vt_lo")
    vt_hi = big.tile([P, NM // 2, C + 1], BF16, tag="x1", name="vt_hi")

    def vt(m):
        return vt_lo[:, m] if m < NM // 2 else vt_hi[:, m - NM // 2]

    for m in range(NM):
        ps = ps_s.tile([P, C], F32, tag="s")
        for ct in range(NCT):
            nc.tensor.matmul(
                ps,
                lhsT=h_sb[ct][:, m * P:(m + 1) * P],
                rhs=wq_sb[ct][:, 2 * C:3 * C],
                start=(ct == 0), stop=(ct == NCT - 1))
        nc.vector.tensor_add(vt(m)[:, 0:C], ps, bv_bc)
        nc.vector.tensor_copy(vt(m)[:, C:C + 1], ones_col)

    # ---- attention + proj + residual, per 512-column block ----
    def emit_div_proj(pend):
        pav0, pav1, psum, nb = pend
        nsl = slice(nb * NBLK, (nb + 1) * NBLK)
        # 1/rowsum; broadcast partition 0 to all 128 partitions via DMA
        recip = work2.tile([1, NBLK], F32, tag="recip")
        nc.vector.reciprocal(recip, psum)
        bc_sb = work2.tile([P, NBLK], F32, tag="bc")
        nc.gpsimd.partition_broadcast(bc_sb, recip)
        att = []
        for ct, pav in ((0, pav0), (1, pav1)):
            a = att_pool.tile([P, NBLK], BF16, tag=f"att{ct}", name=f"att{ct}")
            nc.vector.tensor_mul(a, pav, bc_sb)
            att.append(a)
        for o in range(NCT):
            pp = ps_s.tile([P, NBLK], F32, tag="s")
            for ct in range(NCT):
                nc.tensor.matmul(
                    pp, lhsT=wp_sb[ct][:, o * P:(o + 1) * P],
                    rhs=att[ct], start=(ct == 0), stop=(ct == NCT - 1))
            xres = xres_pool.tile([P, NBLK], F32, tag="xr")
            nc.sync.dma_start(xres, x_d[o * P:(o + 1) * P, nsl])
            st = stage.tile([P, NBLK], F32, tag="st")
            nc.vector.tensor_scalar_add(st, pp, bp_t[o])
            nc.vector.tensor_add(st, st, xres)
            nc.sync.dma_start(out_d[o * P:(o + 1) * P, nsl], st)

    # m-loop with 2-deep scores lookahead: PE program order is
    # s(0) s(1) [div/proj of prev block] s(2) av(0) s(3) av(1) ... so the
    # exp(m) ACT latency is hidden behind s(m+1)/s(m+2) instead of stalling
    # the av(m) matmuls (matmul waits break the PE fill/drain overlap).
    pend = None
    for nb in range(NB):
        nsl = slice(nb * NBLK, (nb + 1) * NBLK)
        qs = [q_sb[ct][:, nsl] for ct in range(NCT)]  # noqa
        pav0 = ps_av0.tile([P, NBLK], F32, tag="av0")
        pav1 = ps_av1.tile([P, NBLK], F32, tag="av1")

        ps_m = {}
        e_m = {}

        def emit_scores(m, qs=qs):
            ps = ps_s.tile([P, NBLK], F32, tag="s")
            for ct in range(NCT):
                nc.tensor.matmul(
                    ps, lhsT=k_sb[ct][:, m * P:(m + 1) * P],
                    rhs=qs[ct], start=(ct == 0), stop=(ct == NCT - 1))
            ps_m[m] = ps

        emit_scores(0)
        emit_scores(1)
        if pend is not None:
            emit_div_proj(pend)
        psum = ps_sum.tile([1, NBLK], F32, tag="sum")
        for m in range(NM):
            if m + 2 < NM:
                emit_scores(m + 2)
            e = work.tile([P, NBLK], BF16, tag="e")
            nc.scalar.activation(e, ps_m.pop(m), AF.Exp, scale=float(SCALE))
            er = e[:]
            first, last = (m == 0), (m == NM - 1)
            vtm = vt(m)
            nc.tensor.matmul(pav0, lhsT=vtm[:, 0:P], rhs=er,
                             start=first, stop=last)
            nc.tensor.matmul(pav1, lhsT=vtm[:, P:2 * P], rhs=er,
                             start=first, stop=last)
            nc.tensor.matmul(psum, lhsT=vtm[:, 2 * P:2 * P + 1],
                             rhs=er, start=first, stop=last)
        pend = (pav0, pav1, psum, nb)
    emit_div_proj(pend)


def build_nc() -> bass.Bass:
    nc = bacc.Bacc("TRN2", target_bir_lowering=False, debug=False)
    x = nc.dram_tensor("x", [C, N], F32, kind="ExternalInput")
    wqkvT = nc.dram_tensor("wqkvT", [C, 3 * C], F32, kind="ExternalInput")
    bqkv = nc.dram_tensor("bqkv", [3 * C], F32, kind="ExternalInput")
    wprojT = nc.dram_tensor("wprojT", [C, C], F32, kind="ExternalInput")
    bproj = nc.dram_tensor("bproj", [C], F32, kind="ExternalInput")
    gamma = nc.dram_tensor("gamma", [C], F32, kind="ExternalInput")
    beta = nc.dram_tensor("beta", [C], F32, kind="ExternalInput")
    gmat = nc.dram_tensor("gmat", [P, P], F32, kind="ExternalInput")
    out = nc.dram_tensor("out", [C, N], F32, kind="ExternalOutput")
    with tile.TileContext(nc) as tc:
        emit_kernel(tc, out.ap(), x.ap(), wqkvT.ap(), bqkv.ap(), wprojT.ap(),
                    bproj.ap(), gamma.ap(), beta.ap(), gmat.ap())
    nc.compile()
    return nc


_NC_CACHE: list = []


def _in_maps(x, gamma, beta, w_qkv, b_qkv, w_proj, b_proj):
    f = lambda a: np.ascontiguousarray(np.asarray(a, dtype=np.float32))
    xs = f(x).reshape(B, C, N)
    base = {
        "wqkvT": f(np.asarray(w_qkv, dtype=np.float32).T),
        "bqkv": f(b_qkv),
        "wprojT": f(np.asarray(w_proj, dtype=np.float32).T),
        "bproj": f(b_proj),
        "gamma": f(gamma),
        "beta": f(beta),
        "gmat": _group_mat(),
    }
    return [{**base, "x": np.ascontiguousarray(xs[i])} for i in range(B)]


def run_spmd(x, gamma, beta, w_qkv, b_qkv, w_proj, b_proj, **kwargs):
    from concourse.bass_utils import run_bass_kernel_spmd

    if not _NC_CACHE:
        _NC_CACHE.append(build_nc())
    nc = _NC_CACHE[0]
    maps = _in_maps(x, gamma, beta, w_qkv, b_qkv, w_proj, b_proj)
    res = run_bass_kernel_spmd(nc, maps, core_ids=list(range(B)), **kwargs)
    out = np.stack([res.results[i]["out"] for i in range(B)])
    return out.reshape(B, C, H, W), res


def kernel(x, gamma, beta, w_qkv, b_qkv, w_proj, b_proj) -> np.ndarray:
    out, _ = run_spmd(x, gamma, beta, w_qkv, b_qkv, w_proj, b_proj)
    return out


# revision 16
# speedup vs baseline: 1.7327x; 1.4982x over previous
"""AttentionBlock (GroupNorm + single-head self-attention + proj + residual)
on 8 TRN2 NeuronCores. Data-parallel over batch: core i handles sample i.

Reference computation per sample (C=256, H=W=64, N=H*W=4096, G=32 groups):
  h    = groupnorm(x) * gamma + beta
  qkv  = w_qkv @ h + b_qkv              (1x1 conv == channel matmul)
  attn = softmax(q^T k / sqrt(C))       (N x N, never materialized in HBM)
  out  = x + w_proj @ (v @ attn^T) + b_proj

Kernel layout choices:
  - h, q, k as (C on partitions, N free) sbuf tensors (2 tiles of 128 chans).
  - v computed directly transposed (N on partitions, C free) with an extra
    ones column, so softmax denominators fall out of the same PE matmuls
    that compute attn @ v (flash-attention style, scores kept transposed).
  - scores^T tile [128 m, 512 n] -> exp on ACT -> 3 accumulating matmuls.
  - softmax never needs a max-subtraction: scores ~ N(0, 0.4^2).
  - matmuls in bf16 (1 cycle/row; f32r is a 2-pass mode on this HW). The
    residual connection dilutes attention-path rounding ~50x, so bf16 keeps
    the end-to-end rel err ~1e-3.
  - division/proj/store for block nb is emitted after block nb+1's m-loop
    (software pipelining) so PE never stalls on the softmax tail.
"""

import sys

for _p in ("/opt/trn_rl_repo", "/opt/pypackages"):
    if _p not in sys.path:
        sys.path.append(_p)

from contextlib import ExitStack

import numpy as np

import concourse.bass as bass
import concourse.tile as tile
from concourse import bacc, mybir
from concourse._compat import with_exitstack

B, C, H, W = 8, 256, 64, 64
N = H * W          # 4096
G = 32             # groups
GS = C // G        # 8 channels per group
EPS = 1e-5
P = 128
NCT = C // P       # 2 channel tiles
NBLK = 512         # attention n-block width
NB = N // NBLK     # 8
NM = N // P        # 32 m-tiles
SCALE = 1.0 / np.sqrt(np.float32(C))  # 1/16
WARMUP_MM = 80     # fp32 gmat matmuls to keep PE's HAM clock-gate warm

F32 = mybir.dt.float32
BF16 = mybir.dt.bfloat16
FP8 = mybir.dt.float8e4
DR = mybir.MatmulPerfMode.DoubleRow
AF = mybir.ActivationFunctionType
ALU = mybir.AluOpType


def _group_mat() -> np.ndarray:
    """A[c, c'] = 1/GS if c and c' are in the same group (within a 128-chan tile).

    out = A^T @ t averages per-channel stats over each group and broadcasts the
    group value back to every channel of the group, in one PE matmul."""
    a = np.zeros((P, P), np.float32)
    for g in range(P // GS):
        a[g * GS:(g + 1) * GS, g * GS:(g + 1) * GS] = 1.0 / GS
    return a


def _col(ap_1d, lo, hi):
    """Slice a 1-D DRAM AP into a [hi-lo, 1] AP (partition dim x 1)."""
    sl = ap_1d[lo:hi]
    return bass.AP(tensor=sl.tensor, offset=sl.offset, ap=[*sl.ap, [1, 1]])


def _bcast_rows(ap_1d, lo, hi, nrows):
    """Read ap_1d[lo:hi] identically into nrows partitions."""
    sl = ap_1d[lo:hi]
    return bass.AP(tensor=sl.tensor, offset=sl.offset, ap=[[0, nrows], *sl.ap])


@with_exitstack
def emit_kernel(ctx: ExitStack, tc: tile.TileContext, out_d, x_d, wqkvT_d,
                bqkv_d, wprojT_d, bproj_d, gamma_d, beta_d, gmat_d):
    nc = tc.nc

    big = ctx.enter_context(tc.tile_pool(name="big", bufs=1))
    small = ctx.enter_context(tc.tile_pool(name="small", bufs=1))
    work = ctx.enter_context(tc.tile_pool(name="work", bufs=3))
    work2 = ctx.enter_context(tc.tile_pool(name="work2", bufs=2))
    att_pool = ctx.enter_context(tc.tile_pool(name="att", bufs=2))
    stage = ctx.enter_context(tc.tile_pool(name="stage", bufs=3))
    xres_pool = ctx.enter_context(tc.tile_pool(name="xres", bufs=3))
    ps_s = ctx.enter_context(tc.tile_pool(name="ps_s", bufs=3, space="PSUM"))
    ps_av0 = ctx.enter_context(tc.tile_pool(name="ps_av0", bufs=2, space="PSUM"))
    ps_av1 = ctx.enter_context(tc.tile_pool(name="ps_av1", bufs=2, space="PSUM"))
    ps_sum = ctx.enter_context(tc.tile_pool(name="ps_sum", bufs=1, space="PSUM"))

    # ---- gmat first: its DVE copy feeds PE warmup matmuls that keep the
    # HAM clock-gate warm while x loads / groupnorm stats run ----
    gmat_f = small.tile([P, P], F32, tag="gmatf")
    nc.sync.dma_start(gmat_f, gmat_d[:, :])
    gmat_sb = small.tile([P, P], F32, tag="gmat")
    nc.vector.tensor_copy(gmat_sb, gmat_f)
    for w in range(WARMUP_MM):
        pw = ps_s.tile([P, P], F32, tag="s", name=f"warm{w}")
        nc.tensor.matmul(pw, lhsT=gmat_sb, rhs=gmat_sb, start=True, stop=True)

    # ---- constants / weights to SBUF ----
    wq_sb = []
    wp_sb = []
    gamma_t = []
    beta_t = []
    for ct in range(NCT):
        wqf = small.tile([P, 3 * C], F32, tag=f"wqkvTf{ct}", name=f"wqf{ct}")
        nc.sync.dma_start(wqf, wqkvT_d[ct * P:(ct + 1) * P, :])
        wq = small.tile([P, 3 * C], BF16, tag=f"wqkvT{ct}", name=f"wq{ct}")
        nc.vector.tensor_copy(wq, wqf)
        wq_sb.append(wq)
        wpf = small.tile([P, C], F32, tag=f"wprojTf{ct}", name=f"wpf{ct}")
        nc.sync.dma_start(wpf, wprojT_d[ct * P:(ct + 1) * P, :])
        wp = small.tile([P, C], BF16, tag=f"wprojT{ct}", name=f"wp{ct}")
        nc.vector.tensor_copy(wp, wpf)
        wp_sb.append(wp)
        gt = small.tile([P, 1], F32, tag=f"gamma{ct}")
        nc.sync.dma_start(gt, _col(gamma_d, ct * P, (ct + 1) * P))
        gamma_t.append(gt)
        bt = small.tile([P, 1], F32, tag=f"beta{ct}")
        nc.sync.dma_start(bt, _col(beta_d, ct * P, (ct + 1) * P))
        beta_t.append(bt)

    bq_t = []
    for o in range(4):  # q, k output-channel tiles
        t = small.tile([P, 1], F32, tag=f"bq{o}")
        nc.sync.dma_start(t, _col(bqkv_d, o * P, (o + 1) * P))
        bq_t.append(t)
    bp_t = []
    for o in range(NCT):
        t = small.tile([P, 1], F32, tag=f"bp{o}")
        nc.sync.dma_start(t, _col(bproj_d, o * P, (o + 1) * P))
        bp_t.append(t)
    bv_bc = small.tile([P, C], F32, tag="bv_bc")
    nc.gpsimd.dma_start(bv_bc, _bcast_rows(bqkv_d, 2 * C, 3 * C, P))

    ones_col = small.tile([P, 1], F32, tag="ones_col")
    nc.vector.memset(ones_col, 1.0)
    eps_t = small.tile([P, 1], F32, tag="eps")
    nc.vector.memset(eps_t, float(EPS))

    # ---- load x (ct0 on the HW DGE queue, ct1 on the SW DGE queue, in
    # parallel); groupnorm stats interleaved with chunk arrival ----
    x_sb = []
    stats_t = []
    for ct in range(NCT):
        xt = big.tile([P, N], F32, tag=f"x{ct}", name=f"x{ct}")
        x_sb.append(xt)
        stats_t.append(small.tile([P, NB, 6], F32, tag=f"bnst{ct}",
                                  name=f"bnst{ct}"))
    for j in range(NB):
        for ct in range(NCT):
            eng = nc.sync if ct == 0 else nc.gpsimd
            eng.dma_start(x_sb[ct][:, j * NBLK:(j + 1) * NBLK],
                          x_d[ct * P:(ct + 1) * P, j * NBLK:(j + 1) * NBLK])
    for j in range(NB):
        for ct in range(NCT):
            nc.vector.bn_stats(stats_t[ct][:, j, :],
                               x_sb[ct][:, j * NBLK:(j + 1) * NBLK])

    h_sb = []
    for ct in range(NCT):
        xt = x_sb[ct]
        mv = small.tile([P, 2], F32, tag=f"mv{ct}")
        nc.vector.bn_aggr(mv, stats_t[ct])
        # t = [mean_c, E[x^2]_c]
        t = small.tile([P, 2], F32, tag=f"t{ct}")
        nc.vector.tensor_copy(t[:, 0:1], mv[:, 0:1])
        nc.vector.tensor_mul(t[:, 1:2], mv[:, 0:1], mv[:, 0:1])
        nc.vector.tensor_add(t[:, 1:2], t[:, 1:2], mv[:, 1:2])
        # group-average + broadcast back to channels via PE
        psg = ps_s.tile([P, 2], F32, tag="s")
        nc.tensor.matmul(psg, lhsT=gmat_sb, rhs=t, start=True, stop=True)
        g_sb = small.tile([P, 2], F32, tag=f"g{ct}")
        nc.vector.tensor_copy(g_sb, psg)
        # scale = gamma * rsqrt(var + eps);  shift = beta - group_mean * scale
        tmp = small.tile([P, 1], F32, tag=f"tmp{ct}")
        sc = small.tile([P, 1], F32, tag=f"sc{ct}")
        sh = small.tile([P, 1], F32, tag=f"sh{ct}")
        nc.vector.tensor_mul(tmp, g_sb[:, 0:1], g_sb[:, 0:1])
        nc.vector.tensor_tensor(tmp, g_sb[:, 1:2], tmp, ALU.subtract)  # var
        nc.scalar.activation(tmp, tmp, AF.Sqrt, bias=eps_t)
        nc.vector.reciprocal(tmp, tmp)                                 # rstd
        nc.vector.tensor_mul(sc, tmp, gamma_t[ct])
        nc.vector.tensor_mul(tmp, g_sb[:, 0:1], sc)
        nc.vector.tensor_tensor(sh, beta_t[ct], tmp, ALU.subtract)
        h = big.tile([P, N], BF16, tag=f"h{ct}", name=f"h{ct}")
        for j in range(4):
            csl = slice(j * (N // 4), (j + 1) * (N // 4))
            nc.vector.tensor_scalar(h[:, csl], xt[:, csl], sc, sh,
                                    op0=ALU.mult, op1=ALU.add)
        h_sb.append(h)

    # ---- qkv projections. q/k land in fp8 [128, 2, N] (channel-half on the
    # middle dim) and v in fp8 m-pair-interleaved [128, 2, 272] tiles so the
    # attention matmuls can use fp8 DoubleRow (2 values/PE-cell -> one 216ns
    # matmul contracts 256). The residual path keeps everything well inside
    # the rel-err budget. ----
    q2 = big.tile([P, 2, N], FP8, tag="q2")
    k2 = big.tile([P, 2, N], FP8, tag="k2")
    for o in range(4):
        dst, j = (q2, o) if o < 2 else (k2, o - 2)
        for blk in range(NB):
            ps = ps_s.tile([P, NBLK], F32, tag="s")
            for ct in range(NCT):
                nc.tensor.matmul(
                    ps,
                    lhsT=wq_sb[ct][:, o * P:(o + 1) * P],
                    rhs=h_sb[ct][:, blk * NBLK:(blk + 1) * NBLK],
                    start=(ct == 0), stop=(ct == NCT - 1))
            nc.vector.tensor_scalar_add(
                dst[:, j, blk * NBLK:(blk + 1) * NBLK], ps, bq_t[o])

    # v, already transposed and m-pair interleaved; col 256 = ones (softmax
    # denominators). 272 = 257 padded so the pair stride is 16-aligned.
    # Reuses the (dead) x tiles' SBUF via shared tags; x is re-read from DRAM
    # later for the residual.
    VTW = 272
    vt_lo = big.tile([P, NM // 4, 2, VTW], FP8, tag="x0", name="vt_lo")
    vt_hi = big.tile([P, NM // 4, 2, VTW], FP8, tag="x1", name="vt_hi")

    def vt2(pair):
        return (vt_lo[:, pair] if pair < NM // 4
                else vt_hi[:, pair - NM // 4])

    for m in range(NM):
        ps = ps_s.tile([P, C], F32, tag="s")
        for ct in range(NCT):
            nc.tensor.matmul(
                ps,
                lhsT=h_sb[ct][:, m * P:(m + 1) * P],
                rhs=wq_sb[ct][:, 2 * C:3 * C],
                start=(ct == 0), stop=(ct == NCT - 1))
        dst = vt2(m // 2)[:, m % 2]
        nc.vector.tensor_add(dst[:, 0:C], ps, bv_bc)
        nc.vector.tensor_copy(dst[:, C:C + 1], ones_col)

    # ---- attention + proj + residual, per 512-column block ----
    def emit_div_proj(pend):
        pav0, pav1, psum, nb = pend
        nsl = slice(nb * NBLK, (nb + 1) * NBLK)
        # 1/rowsum; broadcast partition 0 to all 128 partitions via DMA
        recip = work2.tile([1, NBLK], F32, tag="recip")
        nc.vector.reciprocal(recip, psum)
        bc_sb = work2.tile([P, NBLK], F32, tag="bc")
        nc.gpsimd.partition_broadcast(bc_sb, recip)
        att = []
        for ct, pav in ((0, pav0), (1, pav1)):
            a = att_pool.tile([P, NBLK], BF16, tag=f"att{ct}", name=f"att{ct}")
            nc.vector.tensor_mul(a, pav, bc_sb)
            att.append(a)
        for o in range(NCT):
            pp = ps_s.tile([P, NBLK], F32, tag="s")
            for ct in range(NCT):
                nc.tensor.matmul(
                    pp, lhsT=wp_sb[ct][:, o * P:(o + 1) * P],
                    rhs=att[ct], start=(ct == 0), stop=(ct == NCT - 1))
            xres = xres_pool.tile([P, NBLK], F32, tag="xr")
            nc.sync.dma_start(xres, x_d[o * P:(o + 1) * P, nsl])
            st = stage.tile([P, NBLK], F32, tag="st")
            nc.vector.tensor_scalar_add(st, pp, bp_t[o])
            nc.vector.tensor_add(st, st, xres)
            nc.sync.dma_start(out_d[o * P:(o + 1) * P, nsl], st)

    # m-pair loop, fp8 DoubleRow: one scores matmul contracts all 256
    # channels; av0/av1/sum each consume an m-PAIR per 216ns matmul. Scores
    # for m+2 are emitted between the two exps (half-pair lookahead) so the
    # ACT exp latency stays off the PE critical path. ACT is the pacing
    # engine here (2 exps per pair).
    pend = None
    for nb in range(NB):
        nsl = slice(nb * NBLK, (nb + 1) * NBLK)
        qs = q2[:, :, nsl]
        pav0 = ps_av0.tile([P, NBLK], F32, tag="av0")
        pav1 = ps_av1.tile([P, NBLK], F32, tag="av1")

        ps_m = {}

        def emit_scores(m, qs=qs):
            ps = ps_s.tile([P, NBLK], F32, tag="s")
            nc.tensor.matmul(ps, lhsT=k2[:, :, m * P:(m + 1) * P], rhs=qs,
                             start=True, stop=True, perf_mode=DR)
            ps_m[m] = ps

        emit_scores(0)
        emit_scores(1)
        if pend is not None:
            emit_div_proj(pend)
        psum = ps_sum.tile([1, NBLK], F32, tag="sum")
        for pair in range(NM // 2):
            m0, m1 = 2 * pair, 2 * pair + 1
            e2 = work.tile([P, 2, NBLK], FP8, tag="e")
            nc.scalar.activation(e2[:, 0], ps_m.pop(m0), AF.Exp,
                                 scale=float(SCALE))
            if m0 + 2 < NM:
                emit_scores(m0 + 2)
            nc.scalar.activation(e2[:, 1], ps_m.pop(m1), AF.Exp,
                                 scale=float(SCALE))
            if m1 + 2 < NM:
                emit_scores(m1 + 2)
            first, last = (pair == 0), (pair == NM // 2 - 1)
            vtp = vt2(pair)
            nc.tensor.matmul(pav0, lhsT=vtp[:, :, 0:P], rhs=e2,
                             start=first, stop=last, perf_mode=DR)
            nc.tensor.matmul(pav1, lhsT=vtp[:, :, P:2 * P], rhs=e2,
                             start=first, stop=last, perf_mode=DR)
            nc.tensor.matmul(psum, lhsT=vtp[:, :, 2 * P:2 * P + 1], rhs=e2,
                             start=first, stop=last, perf_mode=DR)
        pend = (pav0, pav1, psum, nb)
    emit_div_proj(pend)


def build_nc() -> bass.Bass:
    nc = bacc.Bacc("TRN2", target_bir_lowering=False, debug=False)
    x = nc.dram_tensor("x", [C, N], F32, kind="ExternalInput")
    wqkvT = nc.dram_tensor("wqkvT", [C, 3 * C], F32, kind="ExternalInput")
    bqkv = nc.dram_tensor("bqkv", [3 * C], F32, kind="ExternalInput")
    wprojT = nc.dram_tensor("wprojT", [C, C], F32, kind="ExternalInput")
    bproj = nc.dram_tensor("bproj", [C], F32, kind="ExternalInput")
    gamma = nc.dram_tensor("gamma", [C], F32, kind="ExternalInput")
    beta = nc.dram_tensor("beta", [C], F32, kind="ExternalInput")
    gmat = nc.dram_tensor("gmat", [P, P], F32, kind="ExternalInput")
    out = nc.dram_tensor("out", [C, N], F32, kind="ExternalOutput")
    with tile.TileContext(nc) as tc:
        emit_kernel(tc, out.ap(), x.ap(), wqkvT.ap(), bqkv.ap(), wprojT.ap(),
                    bproj.ap(), gamma.ap(), beta.ap(), gmat.ap())
    nc.compile()
    return nc


_NC_CACHE: list = []


def _in_maps(x, gamma, beta, w_qkv, b_qkv, w_proj, b_proj):
    f = lambda a: np.ascontiguousarray(np.asarray(a, dtype=np.float32))
    xs = f(x).reshape(B, C, N)
    base = {
        "wqkvT": f(np.asarray(w_qkv, dtype=np.float32).T),
        "bqkv": f(b_qkv),
        "wprojT": f(np.asarray(w_proj, dtype=np.float32).T),
        "bproj": f(b_proj),
        "gamma": f(gamma),
        "beta": f(beta),
        "gmat": _group_mat(),
    }
    return [{**base, "x": np.ascontiguousarray(xs[i])} for i in range(B)]


def run_spmd(x, gamma, beta, w_qkv, b_qkv, w_proj, b_proj, **kwargs):
    from concourse.bass_utils import run_bass_kernel_spmd

    if not _NC_CACHE:
        _NC_CACHE.append(build_nc())
    nc = _NC_CACHE[0]
    maps = _in_maps(x, gamma, beta, w_qkv, b_qkv, w_proj, b_proj)
    res = run_bass_kernel_spmd(nc, maps, core_ids=list(range(B)), **kwargs)
    out = np.stack([res.results[i]["out"] for i in range(B)])
    return out.reshape(B, C, H, W), res


def kernel(x, gamma, beta, w_qkv, b_qkv, w_proj, b_proj) -> np.ndarray:
    out, _ = run_spmd(x, gamma, beta, w_qkv, b_qkv, w_proj, b_proj)
    return out


# revision 17
# speedup vs baseline: 1.8517x; 1.0687x over previous
"""AttentionBlock (GroupNorm + single-head self-attention + proj + residual)
on 8 TRN2 NeuronCores. Data-parallel over batch: core i handles sample i.

Reference computation per sample (C=256, H=W=64, N=H*W=4096, G=32 groups):
  h    = groupnorm(x) * gamma + beta
  qkv  = w_qkv @ h + b_qkv              (1x1 conv == channel matmul)
  attn = softmax(q^T k / sqrt(C))       (N x N, never materialized in HBM)
  out  = x + w_proj @ (v @ attn^T) + b_proj

Kernel layout choices:
  - h, q, k as (C on partitions, N free) sbuf tensors (2 tiles of 128 chans).
  - v computed directly transposed (N on partitions, C free) with an extra
    ones column, so softmax denominators fall out of the same PE matmuls
    that compute attn @ v (flash-attention style, scores kept transposed).
  - scores^T tile [128 m, 512 n] -> exp on ACT -> 3 accumulating matmuls.
  - softmax never needs a max-subtraction: scores ~ N(0, 0.4^2).
  - matmuls in bf16 (1 cycle/row; f32r is a 2-pass mode on this HW). The
    residual connection dilutes attention-path rounding ~50x, so bf16 keeps
    the end-to-end rel err ~1e-3.
  - division/proj/store for block nb is emitted after block nb+1's m-loop
    (software pipelining) so PE never stalls on the softmax tail.
"""

import sys

for _p in ("/opt/trn_rl_repo", "/opt/pypackages"):
    if _p not in sys.path:
        sys.path.append(_p)

from contextlib import ExitStack

import numpy as np

import concourse.bass as bass
import concourse.tile as tile
from concourse import bacc, mybir
from concourse._compat import with_exitstack

B, C, H, W = 8, 256, 64, 64
N = H * W          # 4096
G = 32             # groups
GS = C // G        # 8 channels per group
EPS = 1e-5
P = 128
NCT = C // P       # 2 channel tiles
NBLK = 512         # attention n-block width
NB = N // NBLK     # 8
NM = N // P        # 32 m-tiles
SCALE = 1.0 / np.sqrt(np.float32(C))  # 1/16
WARMUP_MM = 130     # fp32 gmat matmuls to keep PE's HAM clock-gate warm

F32 = mybir.dt.float32
BF16 = mybir.dt.bfloat16
FP8 = mybir.dt.float8e4
DR = mybir.MatmulPerfMode.DoubleRow
AF = mybir.ActivationFunctionType
ALU = mybir.AluOpType


def _group_mat() -> np.ndarray:
    """A[c, c'] = 1/GS if c and c' are in the same group (within a 128-chan tile).

    out = A^T @ t averages per-channel stats over each group and broadcasts the
    group value back to every channel of the group, in one PE matmul."""
    a = np.zeros((P, P), np.float32)
    for g in range(P // GS):
        a[g * GS:(g + 1) * GS, g * GS:(g + 1) * GS] = 1.0 / GS
    return a


def _col(ap_1d, lo, hi):
    """Slice a 1-D DRAM AP into a [hi-lo, 1] AP (partition dim x 1)."""
    sl = ap_1d[lo:hi]
    return bass.AP(tensor=sl.tensor, offset=sl.offset, ap=[*sl.ap, [1, 1]])


def _bcast_rows(ap_1d, lo, hi, nrows):
    """Read ap_1d[lo:hi] identically into nrows partitions."""
    sl = ap_1d[lo:hi]
    return bass.AP(tensor=sl.tensor, offset=sl.offset, ap=[[0, nrows], *sl.ap])


@with_exitstack
def emit_kernel(ctx: ExitStack, tc: tile.TileContext, out_d, x_d, wqkvT_d,
                bqkv_d, wprojT_d, bproj_d, gamma_d, beta_d, gmat_d):
    nc = tc.nc

    big = ctx.enter_context(tc.tile_pool(name="big", bufs=1))
    small = ctx.enter_context(tc.tile_pool(name="small", bufs=1))
    work = ctx.enter_context(tc.tile_pool(name="work", bufs=3))
    work2 = ctx.enter_context(tc.tile_pool(name="work2", bufs=2))
    att_pool = ctx.enter_context(tc.tile_pool(name="att", bufs=2))
    stage = ctx.enter_context(tc.tile_pool(name="stage", bufs=3))
    xres_pool = ctx.enter_context(tc.tile_pool(name="xres", bufs=3))
    ps_s = ctx.enter_context(tc.tile_pool(name="ps_s", bufs=3, space="PSUM"))
    ps_av0 = ctx.enter_context(tc.tile_pool(name="ps_av0", bufs=2, space="PSUM"))
    ps_av1 = ctx.enter_context(tc.tile_pool(name="ps_av1", bufs=2, space="PSUM"))
    ps_sum = ctx.enter_context(tc.tile_pool(name="ps_sum", bufs=1, space="PSUM"))

    # ---- gmat first: its DVE copy feeds PE warmup matmuls that keep the
    # HAM clock-gate warm while x loads / groupnorm stats run ----
    gmat_f = small.tile([P, P], F32, tag="gmatf")
    nc.sync.dma_start(gmat_f, gmat_d[:, :])
    gmat_sb = small.tile([P, P], F32, tag="gmat")
    nc.vector.tensor_copy(gmat_sb, gmat_f)
    for w in range(WARMUP_MM):
        pw = ps_s.tile([P, P], F32, tag="s", name=f"warm{w}")
        nc.tensor.matmul(pw, lhsT=gmat_sb, rhs=gmat_sb, start=True, stop=True)

    # ---- constants / weights to SBUF ----
    wq_sb = []
    wp_sb = []
    gamma_t = []
    beta_t = []
    for ct in range(NCT):
        wqf = small.tile([P, 3 * C], F32, tag=f"wqkvTf{ct}", name=f"wqf{ct}")
        nc.sync.dma_start(wqf, wqkvT_d[ct * P:(ct + 1) * P, :])
        wq = small.tile([P, 3 * C], BF16, tag=f"wqkvT{ct}", name=f"wq{ct}")
        nc.vector.tensor_copy(wq, wqf)
        wq_sb.append(wq)
        wpf = small.tile([P, C], F32, tag=f"wprojTf{ct}", name=f"wpf{ct}")
        nc.sync.dma_start(wpf, wprojT_d[ct * P:(ct + 1) * P, :])
        wp = small.tile([P, C], BF16, tag=f"wprojT{ct}", name=f"wp{ct}")
        nc.vector.tensor_copy(wp, wpf)
        wp_sb.append(wp)
        gt = small.tile([P, 1], F32, tag=f"gamma{ct}")
        nc.sync.dma_start(gt, _col(gamma_d, ct * P, (ct + 1) * P))
        gamma_t.append(gt)
        bt = small.tile([P, 1], F32, tag=f"beta{ct}")
        nc.sync.dma_start(bt, _col(beta_d, ct * P, (ct + 1) * P))
        beta_t.append(bt)

    bq_t = []
    for o in range(4):  # q, k output-channel tiles
        t = small.tile([P, 1], F32, tag=f"bq{o}")
        nc.sync.dma_start(t, _col(bqkv_d, o * P, (o + 1) * P))
        bq_t.append(t)
    bp_t = []
    for o in range(NCT):
        t = small.tile([P, 1], F32, tag=f"bp{o}")
        nc.sync.dma_start(t, _col(bproj_d, o * P, (o + 1) * P))
        bp_t.append(t)
    bv_bc = small.tile([P, C], F32, tag="bv_bc")
    nc.gpsimd.dma_start(bv_bc, _bcast_rows(bqkv_d, 2 * C, 3 * C, P))

    ones_col = small.tile([P, 1], F32, tag="ones_col")
    nc.vector.memset(ones_col, 1.0)
    eps_t = small.tile([P, 1], F32, tag="eps")
    nc.vector.memset(eps_t, float(EPS))

    # ---- load x (ct0 on the HW DGE queue, ct1 on the SW DGE queue, in
    # parallel); groupnorm stats interleaved with chunk arrival ----
    x_sb = []
    stats_t = []
    for ct in range(NCT):
        xt = big.tile([P, N], F32, tag=f"x{ct}", name=f"x{ct}")
        x_sb.append(xt)
        stats_t.append(small.tile([P, NB, 6], F32, tag=f"bnst{ct}",
                                  name=f"bnst{ct}"))
    for j in range(NB):
        for ct in range(NCT):
            eng = nc.sync if ct == 0 else nc.gpsimd
            eng.dma_start(x_sb[ct][:, j * NBLK:(j + 1) * NBLK],
                          x_d[ct * P:(ct + 1) * P, j * NBLK:(j + 1) * NBLK])
    for j in range(NB):
        for ct in range(NCT):
            nc.vector.bn_stats(stats_t[ct][:, j, :],
                               x_sb[ct][:, j * NBLK:(j + 1) * NBLK])

    h_sb = []
    for ct in range(NCT):
        xt = x_sb[ct]
        mv = small.tile([P, 2], F32, tag=f"mv{ct}")
        nc.vector.bn_aggr(mv, stats_t[ct])
        # t = [mean_c, E[x^2]_c]
        t = small.tile([P, 2], F32, tag=f"t{ct}")
        nc.vector.tensor_copy(t[:, 0:1], mv[:, 0:1])
        nc.vector.tensor_mul(t[:, 1:2], mv[:, 0:1], mv[:, 0:1])
        nc.vector.tensor_add(t[:, 1:2], t[:, 1:2], mv[:, 1:2])
        # group-average + broadcast back to channels via PE
        psg = ps_s.tile([P, 2], F32, tag="s")
        nc.tensor.matmul(psg, lhsT=gmat_sb, rhs=t, start=True, stop=True)
        g_sb = small.tile([P, 2], F32, tag=f"g{ct}")
        nc.vector.tensor_copy(g_sb, psg)
        # scale = gamma * rsqrt(var + eps);  shift = beta - group_mean * scale
        tmp = small.tile([P, 1], F32, tag=f"tmp{ct}")
        sc = small.tile([P, 1], F32, tag=f"sc{ct}")
        sh = small.tile([P, 1], F32, tag=f"sh{ct}")
        nc.vector.tensor_mul(tmp, g_sb[:, 0:1], g_sb[:, 0:1])
        nc.vector.tensor_tensor(tmp, g_sb[:, 1:2], tmp, ALU.subtract)  # var
        nc.scalar.activation(tmp, tmp, AF.Sqrt, bias=eps_t)
        nc.vector.reciprocal(tmp, tmp)                                 # rstd
        nc.vector.tensor_mul(sc, tmp, gamma_t[ct])
        nc.vector.tensor_mul(tmp, g_sb[:, 0:1], sc)
        nc.vector.tensor_tensor(sh, beta_t[ct], tmp, ALU.subtract)
        h = big.tile([P, N], BF16, tag=f"h{ct}", name=f"h{ct}")
        for j in range(4):
            csl = slice(j * (N // 4), (j + 1) * (N // 4))
            nc.vector.tensor_scalar(h[:, csl], xt[:, csl], sc, sh,
                                    op0=ALU.mult, op1=ALU.add)
        h_sb.append(h)

    # ---- qkv projections. q/k land in fp8 [128, 2, N] (channel-half on the
    # middle dim) and v in fp8 m-pair-interleaved [128, 2, 272] tiles so the
    # attention matmuls can use fp8 DoubleRow (2 values/PE-cell -> one 216ns
    # matmul contracts 256). The residual path keeps everything well inside
    # the rel-err budget. ----
    q2 = big.tile([P, 2, N], FP8, tag="q2")
    k2 = big.tile([P, 2, N], FP8, tag="k2")
    for o in range(4):
        dst, j = (q2, o) if o < 2 else (k2, o - 2)
        for blk in range(NB):
            ps = ps_s.tile([P, NBLK], F32, tag="s")
            for ct in range(NCT):
                nc.tensor.matmul(
                    ps,
                    lhsT=wq_sb[ct][:, o * P:(o + 1) * P],
                    rhs=h_sb[ct][:, blk * NBLK:(blk + 1) * NBLK],
                    start=(ct == 0), stop=(ct == NCT - 1))
            nc.vector.tensor_scalar_add(
                dst[:, j, blk * NBLK:(blk + 1) * NBLK], ps, bq_t[o])

    # v, already transposed and m-pair interleaved; col 256 = ones (softmax
    # denominators). 272 = 257 padded so the pair stride is 16-aligned.
    # Reuses the (dead) x tiles' SBUF via shared tags; x is re-read from DRAM
    # later for the residual.
    VTW = 272
    vt_lo = big.tile([P, NM // 4, 2, VTW], FP8, tag="x0", name="vt_lo")
    vt_hi = big.tile([P, NM // 4, 2, VTW], FP8, tag="x1", name="vt_hi")

    def vt2(pair):
        return (vt_lo[:, pair] if pair < NM // 4
                else vt_hi[:, pair - NM // 4])

    for m in range(NM):
        ps = ps_s.tile([P, C], F32, tag="s")
        for ct in range(NCT):
            nc.tensor.matmul(
                ps,
                lhsT=h_sb[ct][:, m * P:(m + 1) * P],
                rhs=wq_sb[ct][:, 2 * C:3 * C],
                start=(ct == 0), stop=(ct == NCT - 1))
        dst = vt2(m // 2)[:, m % 2]
        nc.vector.tensor_add(dst[:, 0:C], ps, bv_bc)
        nc.vector.tensor_copy(dst[:, C:C + 1], ones_col)

    # ---- attention + proj + residual, per 512-column block ----
    def emit_div_proj(pend):
        pav0, pav1, psum, nb = pend
        nsl = slice(nb * NBLK, (nb + 1) * NBLK)
        # 1/rowsum; broadcast partition 0 to all 128 partitions via DMA
        recip = work2.tile([1, NBLK], F32, tag="recip")
        nc.vector.reciprocal(recip, psum)
        bc_sb = work2.tile([P, NBLK], F32, tag="bc")
        nc.gpsimd.partition_broadcast(bc_sb, recip)
        att = []
        for ct, pav in ((0, pav0), (1, pav1)):
            a = att_pool.tile([P, NBLK], BF16, tag=f"att{ct}", name=f"att{ct}")
            nc.vector.tensor_mul(a, pav, bc_sb)
            att.append(a)
        for o in range(NCT):
            pp = ps_s.tile([P, NBLK], F32, tag="s")
            for ct in range(NCT):
                nc.tensor.matmul(
                    pp, lhsT=wp_sb[ct][:, o * P:(o + 1) * P],
                    rhs=att[ct], start=(ct == 0), stop=(ct == NCT - 1))
            xres = xres_pool.tile([P, NBLK], F32, tag="xr")
            nc.sync.dma_start(xres, x_d[o * P:(o + 1) * P, nsl])
            st = stage.tile([P, NBLK], F32, tag="st")
            nc.vector.tensor_scalar_add(st, pp, bp_t[o])
            nc.vector.tensor_add(st, st, xres)
            nc.sync.dma_start(out_d[o * P:(o + 1) * P, nsl], st)

    # m-pair loop, fp8 DoubleRow: one scores matmul contracts all 256
    # channels; av0/av1/sum each consume an m-PAIR per 216ns matmul. Scores
    # for m+2 are emitted between the two exps (half-pair lookahead) so the
    # ACT exp latency stays off the PE critical path. ACT is the pacing
    # engine here (2 exps per pair).
    pend = None
    for nb in range(NB):
        nsl = slice(nb * NBLK, (nb + 1) * NBLK)
        qs = q2[:, :, nsl]
        pav0 = ps_av0.tile([P, NBLK], F32, tag="av0")
        pav1 = ps_av1.tile([P, NBLK], F32, tag="av1")

        ps_m = {}

        def emit_scores(m, qs=qs):
            ps = ps_s.tile([P, NBLK], F32, tag="s")
            nc.tensor.matmul(ps, lhsT=k2[:, :, m * P:(m + 1) * P], rhs=qs,
                             start=True, stop=True, perf_mode=DR)
            ps_m[m] = ps

        emit_scores(0)
        emit_scores(1)
        psum = ps_sum.tile([1, NBLK], F32, tag="sum")
        for pair in range(NM // 2):
            m0, m1 = 2 * pair, 2 * pair + 1
            e2 = work.tile([P, 2, NBLK], FP8, tag="e")
            nc.scalar.activation(e2[:, 0], ps_m.pop(m0), AF.Exp,
                                 scale=float(SCALE))
            if m0 + 2 < NM:
                emit_scores(m0 + 2)
            nc.scalar.activation(e2[:, 1], ps_m.pop(m1), AF.Exp,
                                 scale=float(SCALE))
            if m1 + 2 < NM:
                emit_scores(m1 + 2)
            if pair == 5 and pend is not None:
                # emit the previous block's softmax-divide + proj here so the
                # gpsimd partition_broadcast latency hides under this block's
                # m-loop instead of stalling the scores PSUM rotation
                emit_div_proj(pend)
            first, last = (pair == 0), (pair == NM // 2 - 1)
            vtp = vt2(pair)
            nc.tensor.matmul(pav0, lhsT=vtp[:, :, 0:P], rhs=e2,
                             start=first, stop=last, perf_mode=DR)
            nc.tensor.matmul(pav1, lhsT=vtp[:, :, P:2 * P], rhs=e2,
                             start=first, stop=last, perf_mode=DR)
            nc.tensor.matmul(psum, lhsT=vtp[:, :, 2 * P:2 * P + 1], rhs=e2,
                             start=first, stop=last, perf_mode=DR)
        if nb == NB - 1:
            emit_div_proj((pav0, pav1, psum, nb))
        else:
            pend = (pav0, pav1, psum, nb)


def build_nc() -> bass.Bass:
    nc = bacc.Bacc("TRN2", target_bir_lowering=False, debug=False)
    x = nc.dram_tensor("x", [C, N], F32, kind="ExternalInput")
    wqkvT = nc.dram_tensor("wqkvT", [C, 3 * C], F32, kind="ExternalInput")
    bqkv = nc.dram_tensor("bqkv", [3 * C], F32, kind="ExternalInput")
    wprojT = nc.dram_tensor("wprojT", [C, C], F32, kind="ExternalInput")
    bproj = nc.dram_tensor("bproj", [C], F32, kind="ExternalInput")
    gamma = nc.dram_tensor("gamma", [C], F32, kind="ExternalInput")
    beta = nc.dram_tensor("beta", [C], F32, kind="ExternalInput")
    gmat = nc.dram_tensor("gmat", [P, P], F32, kind="ExternalInput")
    out = nc.dram_tensor("out", [C, N], F32, kind="ExternalOutput")
    with tile.TileContext(nc) as tc:
        emit_kernel(tc, out.ap(), x.ap(), wqkvT.ap(), bqkv.ap(), wprojT.ap(),
                    bproj.ap(), gamma.ap(), beta.ap(), gmat.ap())
    nc.compile()
    return nc


_NC_CACHE: list = []


def _in_maps(x, gamma, beta, w_qkv, b_qkv, w_proj, b_proj):
    f = lambda a: np.ascontiguousarray(np.asarray(a, dtype=np.float32))
    xs = f(x).reshape(B, C, N)
    base = {
        "wqkvT": f(np.asarray(w_qkv, dtype=np.float32).T),
        "bqkv": f(b_qkv),
        "wprojT": f(np.asarray(w_proj, dtype=np.float32).T),
        "bproj": f(b_proj),
        "gamma": f(gamma),
        "beta": f(beta),
        "gmat": _group_mat(),
    }
    return [{**base, "x": np.ascontiguousarray(xs[i])} for i in range(B)]


def run_spmd(x, gamma, beta, w_qkv, b_qkv, w_proj, b_proj, **kwargs):
    from concourse.bass_utils import run_bass_kernel_spmd

    if not _NC_CACHE:
        _NC_CACHE.append(build_nc())
    nc = _NC_CACHE[0]
    maps = _in_maps(x, gamma, beta, w_qkv, b_qkv, w_proj, b_proj)
    res = run_bass_kernel_spmd(nc, maps, core_ids=list(range(B)), **kwargs)
    out = np.stack([res.results[i]["out"] for i in range(B)])
    return out.reshape(B, C, H, W), res


def kernel(x, gamma, beta, w_qkv, b_qkv, w_proj, b_proj) -> np.ndarray:
    out, _ = run_spmd(x, gamma, beta, w_qkv, b_qkv, w_proj, b_proj)
    return out


# revision 18
# speedup vs baseline: 1.9342x; 1.0445x over previous
"""AttentionBlock (GroupNorm + single-head self-attention + proj + residual)
on 8 TRN2 NeuronCores. Data-parallel over batch: core i handles sample i.

Reference computation per sample (C=256, H=W=64, N=H*W=4096, G=32 groups):
  h    = groupnorm(x) * gamma + beta
  qkv  = w_qkv @ h + b_qkv              (1x1 conv == channel matmul)
  attn = softmax(q^T k / sqrt(C))       (N x N, never materialized in HBM)
  out  = x + w_proj @ (v @ attn^T) + b_proj

Kernel layout choices:
  - h, q, k as (C on partitions, N free) sbuf tensors (2 tiles of 128 chans).
  - v computed directly transposed (N on partitions, C free) with an extra
    ones column, so softmax denominators fall out of the same PE matmuls
    that compute attn @ v (flash-attention style, scores kept transposed).
  - scores^T tile [128 m, 512 n] -> exp on ACT -> 3 accumulating matmuls.
  - softmax never needs a max-subtraction: scores ~ N(0, 0.4^2).
  - matmuls in bf16 (1 cycle/row; f32r is a 2-pass mode on this HW). The
    residual connection dilutes attention-path rounding ~50x, so bf16 keeps
    the end-to-end rel err ~1e-3.
  - division/proj/store for block nb is emitted after block nb+1's m-loop
    (software pipelining) so PE never stalls on the softmax tail.
"""

import sys

for _p in ("/opt/trn_rl_repo", "/opt/pypackages"):
    if _p not in sys.path:
        sys.path.append(_p)

from contextlib import ExitStack

import numpy as np

import concourse.bass as bass
import concourse.tile as tile
from concourse import bacc, mybir
from concourse._compat import with_exitstack

B, C, H, W = 8, 256, 64, 64
N = H * W          # 4096
G = 32             # groups
GS = C // G        # 8 channels per group
EPS = 1e-5
P = 128
NCT = C // P       # 2 channel tiles
NBLK = 512         # attention n-block width
NB = N // NBLK     # 8
NM = N // P        # 32 m-tiles
SCALE = 1.0 / np.sqrt(np.float32(C))  # 1/16
WARMUP_MM = 130     # fp32 gmat matmuls to keep PE's HAM clock-gate warm

F32 = mybir.dt.float32
BF16 = mybir.dt.bfloat16
FP8 = mybir.dt.float8e4
DR = mybir.MatmulPerfMode.DoubleRow
AF = mybir.ActivationFunctionType
ALU = mybir.AluOpType


def _group_mat() -> np.ndarray:
    """A[c, c'] = 1/GS if c and c' are in the same group (within a 128-chan tile).

    out = A^T @ t averages per-channel stats over each group and broadcasts the
    group value back to every channel of the group, in one PE matmul."""
    a = np.zeros((P, P), np.float32)
    for g in range(P // GS):
        a[g * GS:(g + 1) * GS, g * GS:(g + 1) * GS] = 1.0 / GS
    return a


def _col(ap_1d, lo, hi):
    """Slice a 1-D DRAM AP into a [hi-lo, 1] AP (partition dim x 1)."""
    sl = ap_1d[lo:hi]
    return bass.AP(tensor=sl.tensor, offset=sl.offset, ap=[*sl.ap, [1, 1]])


def _bcast_rows(ap_1d, lo, hi, nrows):
    """Read ap_1d[lo:hi] identically into nrows partitions."""
    sl = ap_1d[lo:hi]
    return bass.AP(tensor=sl.tensor, offset=sl.offset, ap=[[0, nrows], *sl.ap])


@with_exitstack
def emit_kernel(ctx: ExitStack, tc: tile.TileContext, out_d, x_d, wqkvT_d,
                bqkv_d, wprojT_d, bproj_d, gamma_d, beta_d, gmat_d):
    nc = tc.nc

    big = ctx.enter_context(tc.tile_pool(name="big", bufs=1))
    small = ctx.enter_context(tc.tile_pool(name="small", bufs=1))
    work = ctx.enter_context(tc.tile_pool(name="work", bufs=3))
    work2 = ctx.enter_context(tc.tile_pool(name="work2", bufs=2))
    att_pool = ctx.enter_context(tc.tile_pool(name="att", bufs=2))
    stage = ctx.enter_context(tc.tile_pool(name="stage", bufs=3))
    xres_pool = ctx.enter_context(tc.tile_pool(name="xres", bufs=3))
    ps_s = ctx.enter_context(tc.tile_pool(name="ps_s", bufs=3, space="PSUM"))
    ps_av0 = ctx.enter_context(tc.tile_pool(name="ps_av0", bufs=2, space="PSUM"))
    ps_av1 = ctx.enter_context(tc.tile_pool(name="ps_av1", bufs=2, space="PSUM"))
    ps_sum = ctx.enter_context(tc.tile_pool(name="ps_sum", bufs=1, space="PSUM"))

    # ---- gmat first: its DVE copy feeds PE warmup matmuls that keep the
    # HAM clock-gate warm while x loads / groupnorm stats run ----
    gmat_f = small.tile([P, P], F32, tag="gmatf")
    nc.sync.dma_start(gmat_f, gmat_d[:, :])
    gmat_sb = small.tile([P, P], F32, tag="gmat")
    nc.vector.tensor_copy(gmat_sb, gmat_f)
    for w in range(WARMUP_MM):
        pw = ps_s.tile([P, P], F32, tag="s", name=f"warm{w}")
        nc.tensor.matmul(pw, lhsT=gmat_sb, rhs=gmat_sb, start=True, stop=True)

    # ---- constants / weights to SBUF ----
    wq_sb = []
    wp_sb = []
    gamma_t = []
    beta_t = []
    wqf = small.tile([P, 2, 3 * C], F32, tag="wqkvTf", name="wqf")
    nc.sync.dma_start(wqf, wqkvT_d[:, :, :])
    wq2 = small.tile([P, 2, 3 * C], FP8, tag="wqkvT8", name="wq2")
    nc.vector.tensor_copy(wq2, wqf)
    for ct in range(NCT):
        wpf = small.tile([P, C], F32, tag=f"wprojTf{ct}", name=f"wpf{ct}")
        nc.sync.dma_start(wpf, wprojT_d[ct * P:(ct + 1) * P, :])  # noqa
        wp = small.tile([P, C], BF16, tag=f"wprojT{ct}", name=f"wp{ct}")
        nc.vector.tensor_copy(wp, wpf)
        wp_sb.append(wp)
        gt = small.tile([P, 1], F32, tag=f"gamma{ct}")
        nc.sync.dma_start(gt, _col(gamma_d, ct * P, (ct + 1) * P))
        gamma_t.append(gt)
        bt = small.tile([P, 1], F32, tag=f"beta{ct}")
        nc.sync.dma_start(bt, _col(beta_d, ct * P, (ct + 1) * P))
        beta_t.append(bt)

    bq_t = []
    for o in range(4):  # q, k output-channel tiles
        t = small.tile([P, 1], F32, tag=f"bq{o}")
        nc.sync.dma_start(t, _col(bqkv_d, o * P, (o + 1) * P))
        bq_t.append(t)
    bp_t = []
    for o in range(NCT):
        t = small.tile([P, 1], F32, tag=f"bp{o}")
        nc.sync.dma_start(t, _col(bproj_d, o * P, (o + 1) * P))
        bp_t.append(t)
    bv_bc = small.tile([P, C], F32, tag="bv_bc")
    nc.gpsimd.dma_start(bv_bc, _bcast_rows(bqkv_d, 2 * C, 3 * C, P))

    ones_col = small.tile([P, 1], F32, tag="ones_col")
    nc.vector.memset(ones_col, 1.0)
    eps_t = small.tile([P, 1], F32, tag="eps")
    nc.vector.memset(eps_t, float(EPS))

    # ---- load x (ct0 on the HW DGE queue, ct1 on the SW DGE queue, in
    # parallel); groupnorm stats interleaved with chunk arrival ----
    x_sb = []
    stats_t = []
    for ct in range(NCT):
        xt = big.tile([P, N], F32, tag=f"x{ct}", name=f"x{ct}")
        x_sb.append(xt)
        stats_t.append(small.tile([P, NB, 6], F32, tag=f"bnst{ct}",
                                  name=f"bnst{ct}"))
    for j in range(NB):
        for ct in range(NCT):
            eng = nc.sync if ct == 0 else nc.gpsimd
            eng.dma_start(x_sb[ct][:, j * NBLK:(j + 1) * NBLK],
                          x_d[ct * P:(ct + 1) * P, j * NBLK:(j + 1) * NBLK])
    for j in range(NB):
        for ct in range(NCT):
            nc.vector.bn_stats(stats_t[ct][:, j, :],
                               x_sb[ct][:, j * NBLK:(j + 1) * NBLK])

    h2 = big.tile([P, 2, N], FP8, tag="h2")
    for ct in range(NCT):
        xt = x_sb[ct]
        mv = small.tile([P, 2], F32, tag=f"mv{ct}")
        nc.vector.bn_aggr(mv, stats_t[ct])
        # t = [mean_c, E[x^2]_c]
        t = small.tile([P, 2], F32, tag=f"t{ct}")
        nc.vector.tensor_copy(t[:, 0:1], mv[:, 0:1])
        nc.vector.tensor_mul(t[:, 1:2], mv[:, 0:1], mv[:, 0:1])
        nc.vector.tensor_add(t[:, 1:2], t[:, 1:2], mv[:, 1:2])
        # group-average + broadcast back to channels via PE
        psg = ps_s.tile([P, 2], F32, tag="s")
        nc.tensor.matmul(psg, lhsT=gmat_sb, rhs=t, start=True, stop=True)
        g_sb = small.tile([P, 2], F32, tag=f"g{ct}")
        nc.vector.tensor_copy(g_sb, psg)
        # scale = gamma * rsqrt(var + eps);  shift = beta - group_mean * scale
        tmp = small.tile([P, 1], F32, tag=f"tmp{ct}")
        sc = small.tile([P, 1], F32, tag=f"sc{ct}")
        sh = small.tile([P, 1], F32, tag=f"sh{ct}")
        nc.vector.tensor_mul(tmp, g_sb[:, 0:1], g_sb[:, 0:1])
        nc.vector.tensor_tensor(tmp, g_sb[:, 1:2], tmp, ALU.subtract)  # var
        nc.scalar.activation(tmp, tmp, AF.Sqrt, bias=eps_t)
        nc.vector.reciprocal(tmp, tmp)                                 # rstd
        nc.vector.tensor_mul(sc, tmp, gamma_t[ct])
        nc.vector.tensor_mul(tmp, g_sb[:, 0:1], sc)
        nc.vector.tensor_tensor(sh, beta_t[ct], tmp, ALU.subtract)
        for j in range(4):
            csl = slice(j * (N // 4), (j + 1) * (N // 4))
            nc.vector.tensor_scalar(h2[:, ct, csl], xt[:, csl], sc, sh,
                                    op0=ALU.mult, op1=ALU.add)

    # ---- qkv projections. q/k land in fp8 [128, 2, N] (channel-half on the
    # middle dim) and v in fp8 m-pair-interleaved [128, 2, 272] tiles so the
    # attention matmuls can use fp8 DoubleRow (2 values/PE-cell -> one 216ns
    # matmul contracts 256). The residual path keeps everything well inside
    # the rel-err budget. ----
    q2 = big.tile([P, 2, N], FP8, tag="q2")
    k2 = big.tile([P, 2, N], FP8, tag="k2")
    for o in range(4):
        dst, j = (q2, o) if o < 2 else (k2, o - 2)
        for blk in range(NB):
            ps = ps_s.tile([P, NBLK], F32, tag="s")
            nc.tensor.matmul(
                ps, lhsT=wq2[:, :, o * P:(o + 1) * P],
                rhs=h2[:, :, blk * NBLK:(blk + 1) * NBLK],
                start=True, stop=True, perf_mode=DR)
            nc.scalar.activation(
                dst[:, j, blk * NBLK:(blk + 1) * NBLK], ps, AF.Identity,
                bias=bq_t[o], scale=1.0)

    # v, already transposed and m-pair interleaved; col 256 = ones (softmax
    # denominators). 272 = 257 padded so the pair stride is 16-aligned.
    # Reuses the (dead) x tiles' SBUF via shared tags; x is re-read from DRAM
    # later for the residual.
    VTW = 272
    vt_lo = big.tile([P, NM // 4, 2, VTW], FP8, tag="x0", name="vt_lo")
    vt_hi = big.tile([P, NM // 4, 2, VTW], FP8, tag="x1", name="vt_hi")

    def vt2(pair):
        return (vt_lo[:, pair] if pair < NM // 4
                else vt_hi[:, pair - NM // 4])

    for m in range(NM):
        ps = ps_s.tile([P, C], F32, tag="s")
        nc.tensor.matmul(
            ps, lhsT=h2[:, :, m * P:(m + 1) * P],
            rhs=wq2[:, :, 2 * C:3 * C],
            start=True, stop=True, perf_mode=DR)
        dst = vt2(m // 2)[:, m % 2]
        nc.vector.tensor_add(dst[:, 0:C], ps, bv_bc)
        nc.vector.tensor_copy(dst[:, C:C + 1], ones_col)

    # ---- attention + proj + residual, per 512-column block ----
    def emit_div_proj(pend):
        pav0, pav1, psum, nb = pend
        nsl = slice(nb * NBLK, (nb + 1) * NBLK)
        # rowsums: ACT copies them off PSUM (fast bank release), GPSIMD
        # broadcasts to 128 partitions, DVE takes the reciprocal at full
        # partition parallelism (a [1,512] PSUM reciprocal measures 3.3us).
        sums_sb = work2.tile([1, NBLK], F32, tag="sums")
        nc.scalar.activation(sums_sb, psum, AF.Copy, bias=0.0)
        bc2 = work2.tile([P, NBLK], F32, tag="bc2")
        nc.gpsimd.partition_broadcast(bc2, sums_sb)
        bc_sb = work2.tile([P, NBLK], F32, tag="bc")
        nc.vector.reciprocal(bc_sb, bc2)
        att = []
        for ct, pav in ((0, pav0), (1, pav1)):
            a = att_pool.tile([P, NBLK], BF16, tag=f"att{ct}", name=f"att{ct}")
            nc.vector.tensor_mul(a, pav, bc_sb)
            att.append(a)
        for o in range(NCT):
            pp = ps_s.tile([P, NBLK], F32, tag="s")
            for ct in range(NCT):
                nc.tensor.matmul(
                    pp, lhsT=wp_sb[ct][:, o * P:(o + 1) * P],
                    rhs=att[ct], start=(ct == 0), stop=(ct == NCT - 1))
            xres = xres_pool.tile([P, NBLK], F32, tag="xr")
            nc.sync.dma_start(xres, x_d[o * P:(o + 1) * P, nsl])
            st = stage.tile([P, NBLK], F32, tag="st")
            nc.vector.tensor_scalar_add(st, pp, bp_t[o])
            nc.vector.tensor_add(st, st, xres)
            nc.sync.dma_start(out_d[o * P:(o + 1) * P, nsl], st)

    # m-pair loop, fp8 DoubleRow: one scores matmul contracts all 256
    # channels; av0/av1/sum each consume an m-PAIR per 216ns matmul. Scores
    # for m+2 are emitted between the two exps (half-pair lookahead) so the
    # ACT exp latency stays off the PE critical path. ACT is the pacing
    # engine here (2 exps per pair).
    pend = None
    for nb in range(NB):
        nsl = slice(nb * NBLK, (nb + 1) * NBLK)
        qs = q2[:, :, nsl]
        pav0 = ps_av0.tile([P, NBLK], F32, tag="av0")
        pav1 = ps_av1.tile([P, NBLK], F32, tag="av1")

        ps_m = {}

        def emit_scores(m, qs=qs):
            ps = ps_s.tile([P, NBLK], F32, tag="s")
            nc.tensor.matmul(ps, lhsT=k2[:, :, m * P:(m + 1) * P], rhs=qs,
                             start=True, stop=True, perf_mode=DR)
            ps_m[m] = ps

        emit_scores(0)
        emit_scores(1)
        psum = ps_sum.tile([1, NBLK], F32, tag="sum")
        for pair in range(NM // 2):
            m0, m1 = 2 * pair, 2 * pair + 1
            e2 = work.tile([P, 2, NBLK], FP8, tag="e")
            nc.scalar.activation(e2[:, 0], ps_m.pop(m0), AF.Exp,
                                 scale=float(SCALE))
            if m0 + 2 < NM:
                emit_scores(m0 + 2)
            nc.scalar.activation(e2[:, 1], ps_m.pop(m1), AF.Exp,
                                 scale=float(SCALE))
            if m1 + 2 < NM:
                emit_scores(m1 + 2)
            if pair == 5 and pend is not None:
                # emit the previous block's softmax-divide + proj here so the
                # gpsimd partition_broadcast latency hides under this block's
                # m-loop instead of stalling the scores PSUM rotation
                emit_div_proj(pend)
            first, last = (pair == 0), (pair == NM // 2 - 1)
            vtp = vt2(pair)
            nc.tensor.matmul(pav0, lhsT=vtp[:, :, 0:P], rhs=e2,
                             start=first, stop=last, perf_mode=DR)
            nc.tensor.matmul(pav1, lhsT=vtp[:, :, P:2 * P], rhs=e2,
                             start=first, stop=last, perf_mode=DR)
            nc.tensor.matmul(psum, lhsT=vtp[:, :, 2 * P:2 * P + 1], rhs=e2,
                             start=first, stop=last, perf_mode=DR)
        if nb == NB - 1:
            emit_div_proj((pav0, pav1, psum, nb))
        else:
            pend = (pav0, pav1, psum, nb)


def build_nc() -> bass.Bass:
    nc = bacc.Bacc("TRN2", target_bir_lowering=False, debug=False)
    x = nc.dram_tensor("x", [C, N], F32, kind="ExternalInput")
    wqkvT = nc.dram_tensor("wqkvT", [P, 2, 3 * C], F32, kind="ExternalInput")
    bqkv = nc.dram_tensor("bqkv", [3 * C], F32, kind="ExternalInput")
    wprojT = nc.dram_tensor("wprojT", [C, C], F32, kind="ExternalInput")
    bproj = nc.dram_tensor("bproj", [C], F32, kind="ExternalInput")
    gamma = nc.dram_tensor("gamma", [C], F32, kind="ExternalInput")
    beta = nc.dram_tensor("beta", [C], F32, kind="ExternalInput")
    gmat = nc.dram_tensor("gmat", [P, P], F32, kind="ExternalInput")
    out = nc.dram_tensor("out", [C, N], F32, kind="ExternalOutput")
    with tile.TileContext(nc) as tc:
        emit_kernel(tc, out.ap(), x.ap(), wqkvT.ap(), bqkv.ap(), wprojT.ap(),
                    bproj.ap(), gamma.ap(), beta.ap(), gmat.ap())
    nc.compile()
    return nc


_NC_CACHE: list = []


def _in_maps(x, gamma, beta, w_qkv, b_qkv, w_proj, b_proj):
    f = lambda a: np.ascontiguousarray(np.asarray(a, dtype=np.float32))
    xs = f(x).reshape(B, C, N)
    base = {
        "wqkvT": f(np.asarray(w_qkv, dtype=np.float32).T.reshape(2, P, 3 * C).transpose(1, 0, 2)),
        "bqkv": f(b_qkv),
        "wprojT": f(np.asarray(w_proj, dtype=np.float32).T),
        "bproj": f(b_proj),
        "gamma": f(gamma),
        "beta": f(beta),
        "gmat": _group_mat(),
    }
    return [{**base, "x": np.ascontiguousarray(xs[i])} for i in range(B)]


def run_spmd(x, gamma, beta, w_qkv, b_qkv, w_proj, b_proj, **kwargs):
    from concourse.bass_utils import run_bass_kernel_spmd

    if not _NC_CACHE:
        _NC_CACHE.append(build_nc())
    nc = _NC_CACHE[0]
    maps = _in_maps(x, gamma, beta, w_qkv, b_qkv, w_proj, b_proj)
    res = run_bass_kernel_spmd(nc, maps, core_ids=list(range(B)), **kwargs)
    out = np.stack([res.results[i]["out"] for i in range(B)])
    return out.reshape(B, C, H, W), res


def kernel(x, gamma, beta, w_qkv, b_qkv, w_proj, b_proj) -> np.ndarray:
    out, _ = run_spmd(x, gamma, beta, w_qkv, b_qkv, w_proj, b_proj)
    return out


# revision 19
# speedup vs baseline: 1.9509x; 1.0086x over previous
"""AttentionBlock (GroupNorm + single-head self-attention + proj + residual)
on 8 TRN2 NeuronCores. Data-parallel over batch: core i handles sample i.

Reference computation per sample (C=256, H=W=64, N=H*W=4096, G=32 groups):
  h    = groupnorm(x) * gamma + beta
  qkv  = w_qkv @ h + b_qkv              (1x1 conv == channel matmul)
  attn = softmax(q^T k / sqrt(C))       (N x N, never materialized in HBM)
  out  = x + w_proj @ (v @ attn^T) + b_proj

Kernel layout choices:
  - h, q, k as (C on partitions, N free) sbuf tensors (2 tiles of 128 chans).
  - v computed directly transposed (N on partitions, C free) with an extra
    ones column, so softmax denominators fall out of the same PE matmuls
    that compute attn @ v (flash-attention style, scores kept transposed).
  - scores^T tile [128 m, 512 n] -> exp on ACT -> 3 accumulating matmuls.
  - softmax never needs a max-subtraction: scores ~ N(0, 0.4^2).
  - matmuls in bf16 (1 cycle/row; f32r is a 2-pass mode on this HW). The
    residual connection dilutes attention-path rounding ~50x, so bf16 keeps
    the end-to-end rel err ~1e-3.
  - division/proj/store for block nb is emitted after block nb+1's m-loop
    (software pipelining) so PE never stalls on the softmax tail.
"""

import sys

for _p in ("/opt/trn_rl_repo", "/opt/pypackages"):
    if _p not in sys.path:
        sys.path.append(_p)

from contextlib import ExitStack

import numpy as np

import concourse.bass as bass
import concourse.tile as tile
from concourse import bacc, mybir
from concourse._compat import with_exitstack

B, C, H, W = 8, 256, 64, 64
N = H * W          # 4096
G = 32             # groups
GS = C // G        # 8 channels per group
EPS = 1e-5
P = 128
NCT = C // P       # 2 channel tiles
NBLK = 512         # attention n-block width
NB = N // NBLK     # 8
NM = N // P        # 32 m-tiles
SCALE = 1.0 / np.sqrt(np.float32(C))  # 1/16
WARMUP_MM = 115     # fp32 gmat matmuls to keep PE's HAM clock-gate warm

F32 = mybir.dt.float32
BF16 = mybir.dt.bfloat16
FP8 = mybir.dt.float8e4
DR = mybir.MatmulPerfMode.DoubleRow
AF = mybir.ActivationFunctionType
ALU = mybir.AluOpType


def _group_mat() -> np.ndarray:
    """A[c, c'] = 1/GS if c and c' are in the same group (within a 128-chan tile).

    out = A^T @ t averages per-channel stats over each group and broadcasts the
    group value back to every channel of the group, in one PE matmul."""
    a = np.zeros((P, P), np.float32)
    for g in range(P // GS):
        a[g * GS:(g + 1) * GS, g * GS:(g + 1) * GS] = 1.0 / GS
    return a


def _col(ap_1d, lo, hi):
    """Slice a 1-D DRAM AP into a [hi-lo, 1] AP (partition dim x 1)."""
    sl = ap_1d[lo:hi]
    return bass.AP(tensor=sl.tensor, offset=sl.offset, ap=[*sl.ap, [1, 1]])


def _bcast_rows(ap_1d, lo, hi, nrows):
    """Read ap_1d[lo:hi] identically into nrows partitions."""
    sl = ap_1d[lo:hi]
    return bass.AP(tensor=sl.tensor, offset=sl.offset, ap=[[0, nrows], *sl.ap])


@with_exitstack
def emit_kernel(ctx: ExitStack, tc: tile.TileContext, out_d, x_d, wqkvT_d,
                bqkv_d, wprojT_d, bproj_d, gamma_d, beta_d, gmat_d):
    nc = tc.nc

    big = ctx.enter_context(tc.tile_pool(name="big", bufs=1))
    small = ctx.enter_context(tc.tile_pool(name="small", bufs=1))
    work = ctx.enter_context(tc.tile_pool(name="work", bufs=3))
    work2 = ctx.enter_context(tc.tile_pool(name="work2", bufs=3))
    att_pool = ctx.enter_context(tc.tile_pool(name="att", bufs=2))
    stage = ctx.enter_context(tc.tile_pool(name="stage", bufs=4))
    xres_pool = ctx.enter_context(tc.tile_pool(name="xres", bufs=4))
    ps_s = ctx.enter_context(tc.tile_pool(name="ps_s", bufs=3, space="PSUM"))
    ps_av0 = ctx.enter_context(tc.tile_pool(name="ps_av0", bufs=2, space="PSUM"))
    ps_av1 = ctx.enter_context(tc.tile_pool(name="ps_av1", bufs=2, space="PSUM"))
    ps_sum = ctx.enter_context(tc.tile_pool(name="ps_sum", bufs=1, space="PSUM"))

    # ---- gmat first: its DVE copy feeds PE warmup matmuls that keep the
    # HAM clock-gate warm while x loads / groupnorm stats run ----
    gmat_f = small.tile([P, P], F32, tag="gmatf")
    nc.sync.dma_start(gmat_f, gmat_d[:, :])
    gmat_sb = small.tile([P, P], F32, tag="gmat")
    nc.vector.tensor_copy(gmat_sb, gmat_f)
    for w in range(WARMUP_MM):
        pw = ps_s.tile([P, P], F32, tag="s", name=f"warm{w}")
        nc.tensor.matmul(pw, lhsT=gmat_sb, rhs=gmat_sb, start=True, stop=True)

    # ---- constants / weights to SBUF ----
    wq_sb = []
    wp_sb = []
    gamma_t = []
    beta_t = []
    wqf = small.tile([P, 2, 3 * C], F32, tag="wqkvTf", name="wqf")
    nc.sync.dma_start(wqf, wqkvT_d[:, :, :])
    wq2 = small.tile([P, 2, 3 * C], FP8, tag="wqkvT8", name="wq2")
    nc.vector.tensor_copy(wq2, wqf)
    for ct in range(NCT):
        wpf = small.tile([P, C], F32, tag=f"wprojTf{ct}", name=f"wpf{ct}")
        nc.sync.dma_start(wpf, wprojT_d[ct * P:(ct + 1) * P, :])  # noqa
        wp = small.tile([P, C], BF16, tag=f"wprojT{ct}", name=f"wp{ct}")
        nc.vector.tensor_copy(wp, wpf)
        wp_sb.append(wp)
        gt = small.tile([P, 1], F32, tag=f"gamma{ct}")
        nc.sync.dma_start(gt, _col(gamma_d, ct * P, (ct + 1) * P))
        gamma_t.append(gt)
        bt = small.tile([P, 1], F32, tag=f"beta{ct}")
        nc.sync.dma_start(bt, _col(beta_d, ct * P, (ct + 1) * P))
        beta_t.append(bt)

    bq_t = []
    for o in range(4):  # q, k output-channel tiles
        t = small.tile([P, 1], F32, tag=f"bq{o}")
        nc.sync.dma_start(t, _col(bqkv_d, o * P, (o + 1) * P))
        bq_t.append(t)
    bp_t = []
    for o in range(NCT):
        t = small.tile([P, 1], F32, tag=f"bp{o}")
        nc.sync.dma_start(t, _col(bproj_d, o * P, (o + 1) * P))
        bp_t.append(t)
    bv_bc = small.tile([P, C], F32, tag="bv_bc")
    nc.gpsimd.dma_start(bv_bc, _bcast_rows(bqkv_d, 2 * C, 3 * C, P))

    ones_col = small.tile([P, 1], F32, tag="ones_col")
    nc.vector.memset(ones_col, 1.0)
    eps_t = small.tile([P, 1], F32, tag="eps")
    nc.vector.memset(eps_t, float(EPS))

    # ---- load x (ct0 on the HW DGE queue, ct1 on the SW DGE queue, in
    # parallel); groupnorm stats interleaved with chunk arrival ----
    x_sb = []
    stats_t = []
    for ct in range(NCT):
        xt = big.tile([P, N], F32, tag=f"x{ct}", name=f"x{ct}")
        x_sb.append(xt)
        stats_t.append(small.tile([P, NB, 6], F32, tag=f"bnst{ct}",
                                  name=f"bnst{ct}"))
    for j in range(NB):
        for ct in range(NCT):
            eng = nc.sync if ct == 0 else nc.gpsimd
            eng.dma_start(x_sb[ct][:, j * NBLK:(j + 1) * NBLK],
                          x_d[ct * P:(ct + 1) * P, j * NBLK:(j + 1) * NBLK])
    for j in range(NB):
        for ct in range(NCT):
            nc.vector.bn_stats(stats_t[ct][:, j, :],
                               x_sb[ct][:, j * NBLK:(j + 1) * NBLK])

    h2 = big.tile([P, 2, N], FP8, tag="h2")
    scale_sh = []
    for ct in range(NCT):
        mv = small.tile([P, 2], F32, tag=f"mv{ct}")
        nc.vector.bn_aggr(mv, stats_t[ct])
        # t = [mean_c, E[x^2]_c]
        t = small.tile([P, 2], F32, tag=f"t{ct}")
        nc.vector.tensor_copy(t[:, 0:1], mv[:, 0:1])
        nc.vector.tensor_mul(t[:, 1:2], mv[:, 0:1], mv[:, 0:1])
        nc.vector.tensor_add(t[:, 1:2], t[:, 1:2], mv[:, 1:2])
        # group-average + broadcast back to channels via PE
        psg = ps_s.tile([P, 2], F32, tag="s")
        nc.tensor.matmul(psg, lhsT=gmat_sb, rhs=t, start=True, stop=True)
        g_sb = small.tile([P, 2], F32, tag=f"g{ct}")
        nc.vector.tensor_copy(g_sb, psg)
        # scale = gamma * rsqrt(var + eps);  shift = beta - group_mean * scale
        tmp = small.tile([P, 1], F32, tag=f"tmp{ct}")
        sc = small.tile([P, 1], F32, tag=f"sc{ct}")
        sh = small.tile([P, 1], F32, tag=f"sh{ct}")
        nc.vector.tensor_mul(tmp, g_sb[:, 0:1], g_sb[:, 0:1])
        nc.vector.tensor_tensor(tmp, g_sb[:, 1:2], tmp, ALU.subtract)  # var
        nc.scalar.activation(tmp, tmp, AF.Sqrt, bias=eps_t)
        nc.vector.reciprocal(tmp, tmp)                                 # rstd
        nc.vector.tensor_mul(sc, tmp, gamma_t[ct])
        nc.vector.tensor_mul(tmp, g_sb[:, 0:1], sc)
        nc.vector.tensor_tensor(sh, beta_t[ct], tmp, ALU.subtract)
        scale_sh.append((sc, sh))
    for j in range(4):
        csl = slice(j * (N // 4), (j + 1) * (N // 4))
        for ct in range(NCT):
            sc, sh = scale_sh[ct]
            nc.vector.tensor_scalar(h2[:, ct, csl], x_sb[ct][:, csl], sc, sh,
                                    op0=ALU.mult, op1=ALU.add)

    # ---- qkv projections. q/k land in fp8 [128, 2, N] (channel-half on the
    # middle dim) and v in fp8 m-pair-interleaved [128, 2, 272] tiles so the
    # attention matmuls can use fp8 DoubleRow (2 values/PE-cell -> one 216ns
    # matmul contracts 256). The residual path keeps everything well inside
    # the rel-err budget. ----
    q2 = big.tile([P, 2, N], FP8, tag="q2")
    k2 = big.tile([P, 2, N], FP8, tag="k2")
    for o in range(4):
        dst, j = (q2, o) if o < 2 else (k2, o - 2)
        for blk in range(NB):
            ps = ps_s.tile([P, NBLK], F32, tag="s")
            nc.tensor.matmul(
                ps, lhsT=wq2[:, :, o * P:(o + 1) * P],
                rhs=h2[:, :, blk * NBLK:(blk + 1) * NBLK],
                start=True, stop=True, perf_mode=DR)
            nc.scalar.activation(
                dst[:, j, blk * NBLK:(blk + 1) * NBLK], ps, AF.Identity,
                bias=bq_t[o], scale=1.0)

    # v, already transposed and m-pair interleaved; col 256 = ones (softmax
    # denominators). 272 = 257 padded so the pair stride is 16-aligned.
    # Reuses the (dead) x tiles' SBUF via shared tags; x is re-read from DRAM
    # later for the residual.
    VTW = 272
    vt_lo = big.tile([P, NM // 4, 2, VTW], FP8, tag="x0", name="vt_lo")
    vt_hi = big.tile([P, NM // 4, 2, VTW], FP8, tag="x1", name="vt_hi")

    def vt2(pair):
        return (vt_lo[:, pair] if pair < NM // 4
                else vt_hi[:, pair - NM // 4])

    for m in range(NM):
        ps = ps_s.tile([P, C], F32, tag="s")
        nc.tensor.matmul(
            ps, lhsT=h2[:, :, m * P:(m + 1) * P],
            rhs=wq2[:, :, 2 * C:3 * C],
            start=True, stop=True, perf_mode=DR)
        dst = vt2(m // 2)[:, m % 2]
        nc.vector.tensor_add(dst[:, 0:C], ps, bv_bc)
        nc.vector.tensor_copy(dst[:, C:C + 1], ones_col)

    # ---- attention + proj + residual, per 512-column block ----
    def emit_div_proj(pend):
        pav0, pav1, psum, nb = pend
        nsl = slice(nb * NBLK, (nb + 1) * NBLK)
        # rowsums: ACT copies them off PSUM (fast bank release), GPSIMD
        # broadcasts to 128 partitions, DVE takes the reciprocal at full
        # partition parallelism (a [1,512] PSUM reciprocal measures 3.3us).
        sums_sb = work2.tile([1, NBLK], F32, tag="sums")
        nc.scalar.activation(sums_sb, psum, AF.Copy, bias=0.0)
        bc2 = work2.tile([P, NBLK], F32, tag="bc2")
        nc.gpsimd.partition_broadcast(bc2, sums_sb)
        bc_sb = work2.tile([P, NBLK], F32, tag="bc")
        nc.vector.reciprocal(bc_sb, bc2)
        att = []
        for ct, pav in ((0, pav0), (1, pav1)):
            a = att_pool.tile([P, NBLK], BF16, tag=f"att{ct}", name=f"att{ct}")
            nc.vector.tensor_mul(a, pav, bc_sb)
            att.append(a)
        for o in range(NCT):
            pp = ps_s.tile([P, NBLK], F32, tag="s")
            for ct in range(NCT):
                nc.tensor.matmul(
                    pp, lhsT=wp_sb[ct][:, o * P:(o + 1) * P],
                    rhs=att[ct], start=(ct == 0), stop=(ct == NCT - 1))
            xres = xres_pool.tile([P, NBLK], F32, tag="xr")
            nc.sync.dma_start(xres, x_d[o * P:(o + 1) * P, nsl])
            st = stage.tile([P, NBLK], F32, tag="st")
            nc.vector.tensor_scalar_add(st, pp, bp_t[o])
            nc.vector.tensor_add(st, st, xres)
            nc.sync.dma_start(out_d[o * P:(o + 1) * P, nsl], st)

    # m-pair loop, fp8 DoubleRow: one scores matmul contracts all 256
    # channels; av0/av1/sum each consume an m-PAIR per 216ns matmul. Scores
    # for m+2 are emitted between the two exps (half-pair lookahead) so the
    # ACT exp latency stays off the PE critical path. ACT is the pacing
    # engine here (2 exps per pair).
    pend = None
    for nb in range(NB):
        nsl = slice(nb * NBLK, (nb + 1) * NBLK)
        qs = q2[:, :, nsl]
        pav0 = ps_av0.tile([P, NBLK], F32, tag="av0")
        pav1 = ps_av1.tile([P, NBLK], F32, tag="av1")

        ps_m = {}

        def emit_scores(m, qs=qs):
            ps = ps_s.tile([P, NBLK], F32, tag="s")
            nc.tensor.matmul(ps, lhsT=k2[:, :, m * P:(m + 1) * P], rhs=qs,
                             start=True, stop=True, perf_mode=DR)
            ps_m[m] = ps

        emit_scores(0)
        emit_scores(1)
        psum = ps_sum.tile([1, NBLK], F32, tag="sum")
        for pair in range(NM // 2):
            m0, m1 = 2 * pair, 2 * pair + 1
            e2 = work.tile([P, 2, NBLK], FP8, tag="e")
            nc.scalar.activation(e2[:, 0], ps_m.pop(m0), AF.Exp,
                                 scale=float(SCALE))
            if m0 + 2 < NM:
                emit_scores(m0 + 2)
            nc.scalar.activation(e2[:, 1], ps_m.pop(m1), AF.Exp,
                                 scale=float(SCALE))
            if m1 + 2 < NM:
                emit_scores(m1 + 2)
            if pair == 5 and pend is not None:
                # emit the previous block's softmax-divide + proj here so the
                # gpsimd partition_broadcast latency hides under this block's
                # m-loop instead of stalling the scores PSUM rotation
                emit_div_proj(pend)
            first, last = (pair == 0), (pair == NM // 2 - 1)
            vtp = vt2(pair)
            nc.tensor.matmul(pav0, lhsT=vtp[:, :, 0:P], rhs=e2,
                             start=first, stop=last, perf_mode=DR)
            nc.tensor.matmul(pav1, lhsT=vtp[:, :, P:2 * P], rhs=e2,
                             start=first, stop=last, perf_mode=DR)
            nc.tensor.matmul(psum, lhsT=vtp[:, :, 2 * P:2 * P + 1], rhs=e2,
                             start=first, stop=last, perf_mode=DR)
        if nb == NB - 1:
            emit_div_proj((pav0, pav1, psum, nb))
        else:
            pend = (pav0, pav1, psum, nb)


def build_nc() -> bass.Bass:
    nc = bacc.Bacc("TRN2", target_bir_lowering=False, debug=False)
    x = nc.dram_tensor("x", [C, N], F32, kind="ExternalInput")
    wqkvT = nc.dram_tensor("wqkvT", [P, 2, 3 * C], F32, kind="ExternalInput")
    bqkv = nc.dram_tensor("bqkv", [3 * C], F32, kind="ExternalInput")
    wprojT = nc.dram_tensor("wprojT", [C, C], F32, kind="ExternalInput")
    bproj = nc.dram_tensor("bproj", [C], F32, kind="ExternalInput")
    gamma = nc.dram_tensor("gamma", [C], F32, kind="ExternalInput")
    beta = nc.dram_tensor("beta", [C], F32, kind="ExternalInput")
    gmat = nc.dram_tensor("gmat", [P, P], F32, kind="ExternalInput")
    out = nc.dram_tensor("out", [C, N], F32, kind="ExternalOutput")
    with tile.TileContext(nc) as tc:
        emit_kernel(tc, out.ap(), x.ap(), wqkvT.ap(), bqkv.ap(), wprojT.ap(),
                    bproj.ap(), gamma.ap(), beta.ap(), gmat.ap())
    nc.compile()
    return nc


_NC_CACHE: list = []


def _in_maps(x, gamma, beta, w_qkv, b_qkv, w_proj, b_proj):
    f = lambda a: np.ascontiguousarray(np.asarray(a, dtype=np.float32))
    xs = f(x).reshape(B, C, N)
    base = {
        "wqkvT": f(np.asarray(w_qkv, dtype=np.float32).T.reshape(2, P, 3 * C).transpose(1, 0, 2)),
        "bqkv": f(b_qkv),
        "wprojT": f(np.asarray(w_proj, dtype=np.float32).T),
        "bproj": f(b_proj),
        "gamma": f(gamma),
        "beta": f(beta),
        "gmat": _group_mat(),
    }
    return [{**base, "x": np.ascontiguousarray(xs[i])} for i in range(B)]


def run_spmd(x, gamma, beta, w_qkv, b_qkv, w_proj, b_proj, **kwargs):
    from concourse.bass_utils import run_bass_kernel_spmd

    if not _NC_CACHE:
        _NC_CACHE.append(build_nc())
    nc = _NC_CACHE[0]
    maps = _in_maps(x, gamma, beta, w_qkv, b_qkv, w_proj, b_proj)
    res = run_bass_kernel_spmd(nc, maps, core_ids=list(range(B)), **kwargs)
    out = np.stack([res.results[i]["out"] for i in range(B)])
    return out.reshape(B, C, H, W), res


def kernel(x, gamma, beta, w_qkv, b_qkv, w_proj, b_proj) -> np.ndarray:
    out, _ = run_spmd(x, gamma, beta, w_qkv, b_qkv, w_proj, b_proj)
    return out


# revision 20
# speedup vs baseline: 2.0294x; 1.0402x over previous
"""AttentionBlock (GroupNorm + single-head self-attention + proj + residual)
on 8 TRN2 NeuronCores. Data-parallel over batch: core i handles sample i.

Reference computation per sample (C=256, H=W=64, N=H*W=4096, G=32 groups):
  h    = groupnorm(x) * gamma + beta
  qkv  = w_qkv @ h + b_qkv              (1x1 conv == channel matmul)
  attn = softmax(q^T k / sqrt(C))       (N x N, never materialized in HBM)
  out  = x + w_proj @ (v @ attn^T) + b_proj

Kernel layout choices:
  - h, q, k as (C on partitions, N free) sbuf tensors (2 tiles of 128 chans).
  - v computed directly transposed (N on partitions, C free) with an extra
    ones column, so softmax denominators fall out of the same PE matmuls
    that compute attn @ v (flash-attention style, scores kept transposed).
  - scores^T tile [128 m, 512 n] -> exp on ACT -> 3 accumulating matmuls.
  - softmax never needs a max-subtraction: scores ~ N(0, 0.4^2).
  - matmuls in bf16 (1 cycle/row; f32r is a 2-pass mode on this HW). The
    residual connection dilutes attention-path rounding ~50x, so bf16 keeps
    the end-to-end rel err ~1e-3.
  - division/proj/store for block nb is emitted after block nb+1's m-loop
    (software pipelining) so PE never stalls on the softmax tail.
"""

import sys

for _p in ("/opt/trn_rl_repo", "/opt/pypackages"):
    if _p not in sys.path:
        sys.path.append(_p)

from contextlib import ExitStack

import numpy as np

import concourse.bass as bass
import concourse.tile as tile
from concourse import bacc, mybir
from concourse._compat import with_exitstack

B, C, H, W = 8, 256, 64, 64
N = H * W          # 4096
G = 32             # groups
GS = C // G        # 8 channels per group
EPS = 1e-5
P = 128
NCT = C // P       # 2 channel tiles
NBLK = 512         # attention n-block width
NB = N // NBLK     # 8
NM = N // P        # 32 m-tiles
SCALE = 1.0 / np.sqrt(np.float32(C))  # 1/16
WARMUP_MM = 115     # fp32 gmat matmuls to keep PE's HAM clock-gate warm

F32 = mybir.dt.float32
BF16 = mybir.dt.bfloat16
FP8 = mybir.dt.float8e4
DR = mybir.MatmulPerfMode.DoubleRow
AF = mybir.ActivationFunctionType
ALU = mybir.AluOpType


def _group_mat() -> np.ndarray:
    """A[c, c'] = 1/GS if c and c' are in the same group (within a 128-chan tile).

    out = A^T @ t averages per-channel stats over each group and broadcasts the
    group value back to every channel of the group, in one PE matmul."""
    a = np.zeros((P, P), np.float32)
    for g in range(P // GS):
        a[g * GS:(g + 1) * GS, g * GS:(g + 1) * GS] = 1.0 / GS
    return a


def _col(ap_1d, lo, hi):
    """Slice a 1-D DRAM AP into a [hi-lo, 1] AP (partition dim x 1)."""
    sl = ap_1d[lo:hi]
    return bass.AP(tensor=sl.tensor, offset=sl.offset, ap=[*sl.ap, [1, 1]])


def _bcast_rows(ap_1d, lo, hi, nrows):
    """Read ap_1d[lo:hi] identically into nrows partitions."""
    sl = ap_1d[lo:hi]
    return bass.AP(tensor=sl.tensor, offset=sl.offset, ap=[[0, nrows], *sl.ap])


@with_exitstack
def emit_kernel(ctx: ExitStack, tc: tile.TileContext, out_d, x_d, wqkvT_d,
                bqkv_d, wprojT_d, bproj_d, gamma_d, beta_d, gmat_d):
    nc = tc.nc

    big = ctx.enter_context(tc.tile_pool(name="big", bufs=1))
    small = ctx.enter_context(tc.tile_pool(name="small", bufs=1))
    work = ctx.enter_context(tc.tile_pool(name="work", bufs=3))
    work2 = ctx.enter_context(tc.tile_pool(name="work2", bufs=3))
    att_pool = ctx.enter_context(tc.tile_pool(name="att", bufs=2))
    stage = ctx.enter_context(tc.tile_pool(name="stage", bufs=4))
    xres_pool = ctx.enter_context(tc.tile_pool(name="xres", bufs=4))
    ps_s = ctx.enter_context(tc.tile_pool(name="ps_s", bufs=3, space="PSUM"))
    ps_av0 = ctx.enter_context(tc.tile_pool(name="ps_av0", bufs=2, space="PSUM"))
    ps_av1 = ctx.enter_context(tc.tile_pool(name="ps_av1", bufs=2, space="PSUM"))
    ps_sum = ctx.enter_context(tc.tile_pool(name="ps_sum", bufs=1, space="PSUM"))

    # ---- gmat first: its DVE copy feeds PE warmup matmuls that keep the
    # HAM clock-gate warm while x loads / groupnorm stats run ----
    gmat_f = small.tile([P, P], F32, tag="gmatf")
    nc.sync.dma_start(gmat_f, gmat_d[:, :])
    gmat_sb = small.tile([P, P], F32, tag="gmat")
    nc.vector.tensor_copy(gmat_sb, gmat_f)
    for w in range(WARMUP_MM):
        pw = ps_s.tile([P, P], F32, tag="s", name=f"warm{w}")
        nc.tensor.matmul(pw, lhsT=gmat_sb, rhs=gmat_sb, start=True, stop=True)

    # ---- constants / weights to SBUF ----
    wq_sb = []
    wp_sb = []
    gamma_t = []
    beta_t = []
    wqf = small.tile([P, 2, 3 * C], F32, tag="wqkvTf", name="wqf")
    nc.sync.dma_start(wqf, wqkvT_d[:, :, :])
    wq2 = small.tile([P, 2, 3 * C], FP8, tag="wqkvT8", name="wq2")
    nc.vector.tensor_copy(wq2, wqf)
    for ct in range(NCT):
        wpf = small.tile([P, C], F32, tag=f"wprojTf{ct}", name=f"wpf{ct}")
        nc.sync.dma_start(wpf, wprojT_d[ct * P:(ct + 1) * P, :])  # noqa
        wp = small.tile([P, C], BF16, tag=f"wprojT{ct}", name=f"wp{ct}")
        nc.vector.tensor_copy(wp, wpf)
        wp_sb.append(wp)
        gt = small.tile([P, 1], F32, tag=f"gamma{ct}")
        nc.sync.dma_start(gt, _col(gamma_d, ct * P, (ct + 1) * P))
        gamma_t.append(gt)
        bt = small.tile([P, 1], F32, tag=f"beta{ct}")
        nc.sync.dma_start(bt, _col(beta_d, ct * P, (ct + 1) * P))
        beta_t.append(bt)

    bq_t = []
    for o in range(4):  # q, k output-channel tiles
        t = small.tile([P, 1], F32, tag=f"bq{o}")
        nc.sync.dma_start(t, _col(bqkv_d, o * P, (o + 1) * P))
        bq_t.append(t)
    bp_t = []
    for o in range(NCT):
        t = small.tile([P, 1], F32, tag=f"bp{o}")
        nc.sync.dma_start(t, _col(bproj_d, o * P, (o + 1) * P))
        bp_t.append(t)
    bv_bc = small.tile([P, C], F32, tag="bv_bc")
    nc.gpsimd.dma_start(bv_bc, _bcast_rows(bqkv_d, 2 * C, 3 * C, P))

    ones_col = small.tile([P, 1], F32, tag="ones_col")
    nc.vector.memset(ones_col, 1.0)
    eps_t = small.tile([P, 1], F32, tag="eps")
    nc.vector.memset(eps_t, float(EPS))

    # ---- load x (ct0 on the HW DGE queue, ct1 on the SW DGE queue, in
    # parallel); groupnorm stats interleaved with chunk arrival ----
    x_sb = []
    stats_t = []
    for ct in range(NCT):
        xt = big.tile([P, N], F32, tag=f"x{ct}", name=f"x{ct}")
        x_sb.append(xt)
        stats_t.append(small.tile([P, NB, 6], F32, tag=f"bnst{ct}",
                                  name=f"bnst{ct}"))
    for j in range(NB):
        for ct in range(NCT):
            eng = nc.sync if ct == 0 else nc.gpsimd
            eng.dma_start(x_sb[ct][:, j * NBLK:(j + 1) * NBLK],
                          x_d[ct * P:(ct + 1) * P, j * NBLK:(j + 1) * NBLK])
    for j in range(NB):
        for ct in range(NCT):
            nc.vector.bn_stats(stats_t[ct][:, j, :],
                               x_sb[ct][:, j * NBLK:(j + 1) * NBLK])

    h2 = big.tile([P, 2, N], FP8, tag="h2")
    scale_sh = []
    for ct in range(NCT):
        mv = small.tile([P, 2], F32, tag=f"mv{ct}")
        nc.vector.bn_aggr(mv, stats_t[ct])
        # t = [mean_c, E[x^2]_c]
        t = small.tile([P, 2], F32, tag=f"t{ct}")
        nc.vector.tensor_copy(t[:, 0:1], mv[:, 0:1])
        nc.vector.tensor_mul(t[:, 1:2], mv[:, 0:1], mv[:, 0:1])
        nc.vector.tensor_add(t[:, 1:2], t[:, 1:2], mv[:, 1:2])
        # group-average + broadcast back to channels via PE
        psg = ps_s.tile([P, 2], F32, tag="s")
        nc.tensor.matmul(psg, lhsT=gmat_sb, rhs=t, start=True, stop=True)
        g_sb = small.tile([P, 2], F32, tag=f"g{ct}")
        nc.vector.tensor_copy(g_sb, psg)
        # scale = gamma * rsqrt(var + eps);  shift = beta - group_mean * scale
        tmp = small.tile([P, 1], F32, tag=f"tmp{ct}")
        sc = small.tile([P, 1], F32, tag=f"sc{ct}")
        sh = small.tile([P, 1], F32, tag=f"sh{ct}")
        nc.vector.tensor_mul(tmp, g_sb[:, 0:1], g_sb[:, 0:1])
        nc.vector.tensor_tensor(tmp, g_sb[:, 1:2], tmp, ALU.subtract)  # var
        nc.scalar.activation(tmp, tmp, AF.Sqrt, bias=eps_t)
        nc.vector.reciprocal(tmp, tmp)                                 # rstd
        nc.vector.tensor_mul(sc, tmp, gamma_t[ct])
        nc.vector.tensor_mul(tmp, g_sb[:, 0:1], sc)
        nc.vector.tensor_tensor(sh, beta_t[ct], tmp, ALU.subtract)
        scale_sh.append((sc, sh))
    for j in range(4):
        csl = slice(j * (N // 4), (j + 1) * (N // 4))
        for ct in range(NCT):
            sc, sh = scale_sh[ct]
            nc.vector.tensor_scalar(h2[:, ct, csl], x_sb[ct][:, csl], sc, sh,
                                    op0=ALU.mult, op1=ALU.add)

    # ---- qkv projections. q/k land in fp8 [128, 2, N] (channel-half on the
    # middle dim) and v in fp8 m-pair-interleaved [128, 2, 272] tiles so the
    # attention matmuls can use fp8 DoubleRow (2 values/PE-cell -> one 216ns
    # matmul contracts 256). The residual path keeps everything well inside
    # the rel-err budget. ----
    q2 = big.tile([P, 2, N], FP8, tag="q2")
    k2 = big.tile([P, 2, N], FP8, tag="k2")
    for o in range(4):
        dst, j = (q2, o) if o < 2 else (k2, o - 2)
        for blk in range(NB):
            ps = ps_s.tile([P, NBLK], F32, tag="s")
            nc.tensor.matmul(
                ps, lhsT=wq2[:, :, o * P:(o + 1) * P],
                rhs=h2[:, :, blk * NBLK:(blk + 1) * NBLK],
                start=True, stop=True, perf_mode=DR)
            nc.scalar.activation(
                dst[:, j, blk * NBLK:(blk + 1) * NBLK], ps, AF.Identity,
                bias=bq_t[o], scale=1.0)

    # v, already transposed and m-pair interleaved; col 256 = ones (softmax
    # denominators). 272 = 257 padded so the pair stride is 16-aligned.
    # Reuses the (dead) x tiles' SBUF via shared tags; x is re-read from DRAM
    # later for the residual.
    VTW = 272
    vt_lo = big.tile([P, NM // 4, 2, VTW], FP8, tag="x0", name="vt_lo")
    vt_hi = big.tile([P, NM // 4, 2, VTW], FP8, tag="x1", name="vt_hi")

    def vt2(pair):
        return (vt_lo[:, pair] if pair < NM // 4
                else vt_hi[:, pair - NM // 4])

    for m in range(NM):
        ps = ps_s.tile([P, C], F32, tag="s")
        nc.tensor.matmul(
            ps, lhsT=h2[:, :, m * P:(m + 1) * P],
            rhs=wq2[:, :, 2 * C:3 * C],
            start=True, stop=True, perf_mode=DR)
        dst = vt2(m // 2)[:, m % 2]
        nc.vector.tensor_add(dst[:, 0:C], ps, bv_bc)
        nc.vector.tensor_copy(dst[:, C:C + 1], ones_col)

    # ---- attention + proj + residual, per 512-column block. The divide /
    # proj for block nb is split: the rowsum -> reciprocal-broadcast chain
    # (ACT copy off PSUM, GPSIMD partition broadcast, DVE reciprocal) starts
    # right at the next block's boundary where ACT has a bubble; the
    # att-muls + proj matmuls + residual/store are emitted a few m-pairs
    # later so the chain latency hides under the m-loop. ----
    def emit_div_a(pend):
        pav0, pav1, psum, nb = pend
        # rowsums: a [1,512] PSUM reciprocal on DVE measures 3.3us, so copy
        # off PSUM on ACT, broadcast on GPSIMD, reciprocal at full width.
        sums_sb = work2.tile([1, NBLK], F32, tag="sums")
        nc.scalar.activation(sums_sb, psum, AF.Copy, bias=0.0)
        bc2 = work2.tile([P, NBLK], F32, tag="bc2")
        nc.gpsimd.partition_broadcast(bc2, sums_sb)
        bc_sb = work2.tile([P, NBLK], F32, tag="bc")
        nc.vector.reciprocal(bc_sb, bc2)
        return bc_sb

    def emit_div_b(pend, bc_sb):
        pav0, pav1, psum, nb = pend
        nsl = slice(nb * NBLK, (nb + 1) * NBLK)
        att = []
        for ct, pav in ((0, pav0), (1, pav1)):
            a = att_pool.tile([P, NBLK], BF16, tag=f"att{ct}", name=f"att{ct}")
            nc.vector.tensor_mul(a, pav, bc_sb)
            att.append(a)
        for o in range(NCT):
            pp = ps_s.tile([P, NBLK], F32, tag="s")
            for ct in range(NCT):
                nc.tensor.matmul(
                    pp, lhsT=wp_sb[ct][:, o * P:(o + 1) * P],
                    rhs=att[ct], start=(ct == 0), stop=(ct == NCT - 1))
            xres = xres_pool.tile([P, NBLK], F32, tag="xr")
            nc.sync.dma_start(xres, x_d[o * P:(o + 1) * P, nsl])
            st = stage.tile([P, NBLK], F32, tag="st")
            nc.vector.tensor_scalar_add(st, pp, bp_t[o])
            nc.vector.tensor_add(st, st, xres)
            nc.sync.dma_start(out_d[o * P:(o + 1) * P, nsl], st)

    # m-pair loop, fp8 DoubleRow: one scores matmul contracts all 256
    # channels; av0/av1/sum each consume an m-PAIR per 216ns matmul. Scores
    # for m+2 are emitted between the two exps (half-pair lookahead) so the
    # ACT exp latency stays off the PE critical path. ACT is the pacing
    # engine here (2 exps per pair).
    pend = None
    for nb in range(NB):
        nsl = slice(nb * NBLK, (nb + 1) * NBLK)
        qs = q2[:, :, nsl]
        pav0 = ps_av0.tile([P, NBLK], F32, tag="av0")
        pav1 = ps_av1.tile([P, NBLK], F32, tag="av1")

        ps_m = {}

        def emit_scores(m, qs=qs):
            ps = ps_s.tile([P, NBLK], F32, tag="s")
            nc.tensor.matmul(ps, lhsT=k2[:, :, m * P:(m + 1) * P], rhs=qs,
                             start=True, stop=True, perf_mode=DR)
            ps_m[m] = ps

        emit_scores(0)
        emit_scores(1)
        bc_prev = emit_div_a(pend) if pend is not None else None
        psum = ps_sum.tile([1, NBLK], F32, tag="sum")
        for pair in range(NM // 2):
            m0, m1 = 2 * pair, 2 * pair + 1
            e2 = work.tile([P, 2, NBLK], FP8, tag="e")
            nc.scalar.activation(e2[:, 0], ps_m.pop(m0), AF.Exp,
                                 scale=float(SCALE))
            if m0 + 2 < NM:
                emit_scores(m0 + 2)
            nc.scalar.activation(e2[:, 1], ps_m.pop(m1), AF.Exp,
                                 scale=float(SCALE))
            if m1 + 2 < NM:
                emit_scores(m1 + 2)
            if pair == 4 and pend is not None:
                emit_div_b(pend, bc_prev)
            first, last = (pair == 0), (pair == NM // 2 - 1)
            vtp = vt2(pair)
            nc.tensor.matmul(pav0, lhsT=vtp[:, :, 0:P], rhs=e2,
                             start=first, stop=last, perf_mode=DR)
            nc.tensor.matmul(pav1, lhsT=vtp[:, :, P:2 * P], rhs=e2,
                             start=first, stop=last, perf_mode=DR)
            nc.tensor.matmul(psum, lhsT=vtp[:, :, 2 * P:2 * P + 1], rhs=e2,
                             start=first, stop=last, perf_mode=DR)
        pend = (pav0, pav1, psum, nb)
    bc_prev = emit_div_a(pend)
    emit_div_b(pend, bc_prev)


def build_nc() -> bass.Bass:
    nc = bacc.Bacc("TRN2", target_bir_lowering=False, debug=False)
    x = nc.dram_tensor("x", [C, N], F32, kind="ExternalInput")
    wqkvT = nc.dram_tensor("wqkvT", [P, 2, 3 * C], F32, kind="ExternalInput")
    bqkv = nc.dram_tensor("bqkv", [3 * C], F32, kind="ExternalInput")
    wprojT = nc.dram_tensor("wprojT", [C, C], F32, kind="ExternalInput")
    bproj = nc.dram_tensor("bproj", [C], F32, kind="ExternalInput")
    gamma = nc.dram_tensor("gamma", [C], F32, kind="ExternalInput")
    beta = nc.dram_tensor("beta", [C], F32, kind="ExternalInput")
    gmat = nc.dram_tensor("gmat", [P, P], F32, kind="ExternalInput")
    out = nc.dram_tensor("out", [C, N], F32, kind="ExternalOutput")
    with tile.TileContext(nc) as tc:
        emit_kernel(tc, out.ap(), x.ap(), wqkvT.ap(), bqkv.ap(), wprojT.ap(),
                    bproj.ap(), gamma.ap(), beta.ap(), gmat.ap())
    nc.compile()
    return nc


_NC_CACHE: list = []


def _in_maps(x, gamma, beta, w_qkv, b_qkv, w_proj, b_proj):
    f = lambda a: np.ascontiguousarray(np.asarray(a, dtype=np.float32))
    xs = f(x).reshape(B, C, N)
    base = {
        "wqkvT": f(np.asarray(w_qkv, dtype=np.float32).T.reshape(2, P, 3 * C).transpose(1, 0, 2)),
        "bqkv": f(b_qkv),
        "wprojT": f(np.asarray(w_proj, dtype=np.float32).T),
        "bproj": f(b_proj),
        "gamma": f(gamma),
        "beta": f(beta),
        "gmat": _group_mat(),
    }
    return [{**base, "x": np.ascontiguousarray(xs[i])} for i in range(B)]


def run_spmd(x, gamma, beta, w_qkv, b_qkv, w_proj, b_proj, **kwargs):
    from concourse.bass_utils import run_bass_kernel_spmd

    if not _NC_CACHE:
        _NC_CACHE.append(build_nc())
    nc = _NC_CACHE[0]
    maps = _in_maps(x, gamma, beta, w_qkv, b_qkv, w_proj, b_proj)
    res = run_bass_kernel_spmd(nc, maps, core_ids=list(range(B)), **kwargs)
    out = np.stack([res.results[i]["out"] for i in range(B)])
    return out.reshape(B, C, H, W), res


def kernel(x, gamma, beta, w_qkv, b_qkv, w_proj, b_proj) -> np.ndarray:
    out, _ = run_spmd(x, gamma, beta, w_qkv, b_qkv, w_proj, b_proj)
    return out
